# revision 61
# baseline (speedup 1.0000x reference)
"""AdaptiveGraphConv Trainium2 kernel — 8-core batch-parallel Bass/Tile.

Math (per sample n):
  xm     = mean_t x[n]                                  [C, V]
  theta  = W_theta @ xm + b_theta ; phi similarly       [E, V]
  Cmat   = softmax_w(theta^T @ phi)                     [V, V]
  adap_s = A[s] + B[s] + Cmat                           [V, V]
  out[n] = sum_s W_eff_s @ x[n] @_v adap_s + b_eff      [Co, T, V]
where W_eff_s[co,c] = sum_sg W_big[sg*Co+co, s*C+c], b_eff = sum_sg b_big[sg*Co:+Co]
(both reductions done on device).

Device dataflow (per core, 4 samples). T*V is split into 60 chunks of
(5t, 25v) = 125 elements; a 126th "bias" column per chunk (host-packed,
1.0 at channel 64) carries b_eff through both matmul steps:
  step1: matmul(lhsT = x chunk [c=65, m=126], rhs = Weff_cat [65, 192])
         -> y chunk [(5t,25v)+bias, (s,co)] in PSUM, groups of 5 chunks
         -> y_sb bf16 (plain slice copies on ACT/DVE/Pool)
  mean : accumulating matmul rhs=I64 over all chunks -> x^T sums
  tiny : mean/theta/phi/sim/softmax -> bd3 [126, 3*125] blockdiag bf16
         (bias row 125 = ones in the s=0 block)
  step2: matmul(lhsT = bd3 s-block [126,125], rhs = y chunk [126, 64co])
         accumulated over s, groups of 8 chunks -> [125, 64] -> o_sb bf16
         -> DMA out (host upcasts)
"""

import numpy as np
import ml_dtypes

N, C, T, V, S, E, Co = 32, 64, 300, 25, 3, 64, 64
CP = C + 1                # 65 = x channels + bias channel
NCORES = 8
NL = N // NCORES          # samples per core = 4
CH = 126                  # chunk partitions: (5t, 25v) + bias column
NCH = 60                  # chunks per sample (T/5)
W2 = S * Co               # 192 = y columns per chunk
XW = NCH * CH             # 7560 = x free size per sample
G1 = 4                    # step1 chunks per psum group (15 groups)
G2 = 8                    # step2 chunks per psum group (7 full + 1 of 4)
CF = 1992                 # packed consts free size

_CACHE = {}


def _import_concourse():
    try:
        import concourse  # noqa: F401
    except ImportError:
        import sys

        for p in ("/opt/trn_rl_repo", "/root/.axon_site/_ro/trn_rl_repo"):
            if p not in sys.path:
                sys.path.insert(0, p)


def _build_nc():
    _import_concourse()
    import concourse.bass as bass
    import concourse.bacc as bacc
    import concourse.mybir as mybir
    from concourse import tile

    dt = mybir.dt
    f32, bf16 = dt.float32, dt.bfloat16
    AX = mybir.AxisListType
    ALU = mybir.AluOpType
    ACTF = mybir.ActivationFunctionType

    nc = bacc.Bacc(None, target_bir_lowering=False)

    x_ext = nc.declare_dram_parameter("x", [NL, CP, XW], bf16, isOutput=False)
    c_ext = nc.declare_dram_parameter("consts", [CH, CF], bf16, isOutput=False)
    out_ext = nc.declare_dram_parameter(
        "out", [NL, 125, NCH * Co], bf16, isOutput=True
    )

    with tile.TileContext(nc) as tc:
        with (
            tc.tile_pool(name="const", bufs=1) as cpool,
            tc.tile_pool(name="xin", bufs=2) as xpool,
            tc.tile_pool(name="y", bufs=2) as ypool,
            tc.tile_pool(name="osb", bufs=2) as opool,
            tc.tile_pool(name="small", bufs=2) as spool,
            tc.tile_pool(name="p1", bufs=2, space="PSUM") as pq,
            tc.tile_pool(name="p2", bufs=2, space="PSUM") as po,
            tc.tile_pool(name="pxs", bufs=1, space="PSUM") as pxs,
            tc.tile_pool(name="ptiny", bufs=1, space="PSUM") as pt,
        ):
            # ---------------- PE p-state warmup ----------------
            # ~72 tiny matmuls bridge the DMA/weight-prep startup so the PE
            # hits its 3us continuous-busy ramp before real work arrives.
            wz = cpool.tile([1, Co], bf16)
            nc.gpsimd.memset(wz[:, :], 0.0)
            # tt: single psum bank shared by warmup + all tiny matmul outs
            tt = pt.tile([CH, 301], f32, tag="tt")
            for _ in range(80):
                nc.tensor.matmul(
                    out=tt[0:1, 0:Co], lhsT=wz[:, 0:1], rhs=wz[:, :],
                    start=True, stop=True,
                )

            # ---------------- constants / weight prep ----------------
            ct = cpool.tile([CH, CF], bf16)
            nc.sync.dma_start(out=ct[0:CP, 0:832], in_=c_ext[0:CP, 0:832])
            # part 2 (obd/bg3/A+B, first needed ~14us in) is issued after
            # sample 0's x slices so it doesn't delay them on the DMA rings
            def consts_p2():
                nc.sync.dma_start(out=ct[:, 832:CF], in_=c_ext[:, 832:CF])
                nc.vector.tensor_copy(out=wtp[:, :], in_=wtpf)
                nc.vector.tensor_copy(out=obd[:, :], in_=obdf)
                nc.vector.tensor_copy(out=selc[:, :], in_=self_f)
                nc.vector.tensor_copy(out=id25f[:, :], in_=ct[0:V, 576:601])
                nc.vector.tensor_copy(out=btpf[:, :], in_=ct[0:E, 960:962])
            wbp = ct[0:C, 0:576]
            id64f = ct[0:C, 576:640]
            bbrow = ct[0:1, 640:832]
            wtpf = ct[0:C, 832:960]
            btt = ct[0:E, 960:961]
            btf = ct[0:E, 961:962]
            obdf = ct[0:CH, 962:987]
            self_f = ct[0:V, 987:1617]
            bgab3f = ct[0:CH, 1617:1992]

            wstack = cpool.tile([CP, W2 + C], bf16)
            nc.gpsimd.memset(wstack[C : C + 1, :], 0.0)
            weff_t = cpool.tile([C, W2], f32)
            nc.vector.tensor_tensor(
                out=weff_t[:, :], in0=wbp[:, 0:192], in1=wbp[:, 192:384], op=ALU.add
            )
            # second add writes the bf16 wstack directly (off the critical
            # path copy); id64 cast runs on Pool in parallel
            nc.vector.tensor_tensor(
                out=wstack[0:C, 0:192], in0=weff_t[:, :],
                in1=wbp[:, 384:576], op=ALU.add,
            )
            nc.gpsimd.tensor_copy(out=wstack[0:C, 192:256], in_=id64f)

            befff = cpool.tile([1, Co], f32)
            nc.gpsimd.tensor_tensor(
                out=befff[:, :], in0=bbrow[:, 0:64], in1=bbrow[:, 64:128], op=ALU.add
            )
            # final add writes the bias row in place (partition 64 is
            # 32-aligned, so a partition-shifted engine write is legal)
            nc.gpsimd.tensor_tensor(
                out=wstack[C : C + 1, 0:Co], in0=befff[:, :],
                in1=bbrow[:, 128:192], op=ALU.add,
            )

            wtp = cpool.tile([C, 2 * E], bf16)
            obd = cpool.tile([CH, V], bf16)
            selc = cpool.tile([V, 5 * CH], bf16)
            id25f = cpool.tile([V, V], f32)
            btpf = cpool.tile([E, 2], f32)

            # ---------------- per-sample phases ----------------
            XTLAG = 4  # xtp matmuls trail y matmuls by this many groups

            def phase_a_start(n):
                """Allocate tiles + x DMA for sample n. Sample 0 is on the
                critical path: its later slices issue from the (idle) ACT
                hwdge queue in parallel with SP so the issue rate doesn't
                gate delivery."""
                x_sb = xpool.tile([CP, XW], bf16, tag="x")
                if n == 0:
                    cuts = [0, 6 * CH, 20 * CH, 40 * CH, XW]
                    qs = [nc.sync, nc.sync, nc.scalar, nc.scalar]
                else:
                    cuts = [0, 30 * CH, XW]
                    qs = [nc.sync, nc.sync]
                for q, (lo, hi) in zip(qs, zip(cuts, cuts[1:])):
                    q.dma_start(
                        out=x_sb[:, lo:hi], in_=x_ext[n][:, lo:hi]
                    )
                y_sb = ypool.tile([CH, NCH * W2], bf16, tag="y")
                xtp = pxs.tile([CH, C], f32, tag="xt")
                return {"x": x_sb, "y": y_sb, "xtp": xtp, "n": n}

            def emit_xt(ctx, lo, hi):
                """x^T accumulation matmuls for chunks [lo, hi)."""
                x_sb, xtp = ctx["x"], ctx["xtp"]
                for ch in range(lo, hi):
                    nc.tensor.matmul(
                        out=xtp[:, :],
                        lhsT=x_sb[:, ch * CH : (ch + 1) * CH],
                        rhs=wstack[:, W2 : W2 + C],
                        start=(ch == 0),
                        stop=(ch == NCH - 1),
                    )

            def emit_group(ctx, g, xt="lag", mid=None):
                """step1 psum group g: 4 y-matmuls (+ lagged xt), 1 copy.
                A matmul out may not cross a 512-f32 psum bank boundary, so
                chunk j sits at column (j//2)*512 + (j%2)*192."""
                x_sb, y_sb = ctx["x"], ctx["y"]
                yp = pq.tile([CH, 1024], f32, tag="p1")
                for j in range(G1):
                    ch = G1 * g + j
                    col = (j // 2) * 512 + (j % 2) * W2
                    nc.tensor.matmul(
                        out=yp[:, col : col + W2],
                        lhsT=x_sb[:, ch * CH : (ch + 1) * CH],
                        rhs=wstack[:, 0:W2],
                        start=(j % 2 == 0),
                        stop=(j % 2 == 1),
                    )
                if xt == "lag":
                    if g >= XTLAG:
                        emit_xt(ctx, (g - XTLAG) * G1, (g - XTLAG + 1) * G1)
                    if g == 14:
                        emit_xt(ctx, (15 - XTLAG) * G1, NCH)
                if mid is not None:
                    mid()  # latency-critical ops enqueue ahead of the copy
                dst = y_sb[
                    :, g * G1 * W2 : (g + 1) * G1 * W2
                ].rearrange("p (b w) -> p b w", w=2 * W2)
                src = yp[:, :].rearrange("p (b w) -> p b w", w=512)[
                    :, :, 0 : 2 * W2
                ]
                # GPSIMD cannot access PSUM (walrus birverifier rule), so
                # psum->sbuf copies are split across ACT and DVE only.
                if g % 2 == 0:  # 8 on ACT
                    nc.scalar.copy(out=dst, in_=src)
                else:  # 7 on DVE
                    nc.vector.tensor_copy(out=dst, in_=src)

            def tiny_steps(ctx):
                """Mean/softmax/bd3 chain as 6 steps; weave each between PE
                group emissions so cross-engine hops never stall the PE."""
                n = ctx["n"]

                def s0():  # xta copy (DVE)
                    xta_sb = spool.tile([CH, C], bf16, tag="xta")
                    nc.vector.tensor_copy(out=xta_sb[:, :], in_=ctx["xtp"])
                    ctx["xta"] = xta_sb

                def s1():  # T-sum matmul, then scale by 1/T -> mean
                    xsp = tt[0:V, 0:C]
                    nc.tensor.matmul(
                        out=xsp, lhsT=obd[:, :], rhs=ctx["xta"][:, :],
                        start=True, stop=True,
                    )
                    xs_sb = spool.tile([V, C], f32, tag="xs_sb")
                    nc.scalar.activation(
                        out=xs_sb[:, :], in_=xsp, func=ACTF.Copy,
                        scale=1.0 / T,
                    )
                    ctx["xs"] = xs_sb

                def s2():  # transpose to [c, v]
                    xmt = tt[0:C, 64:89]
                    nc.tensor.transpose(
                        out=xmt, in_=ctx["xs"], identity=id25f[:, :]
                    )
                    xm_sb = spool.tile([C, V], bf16, tag="xm_sb")
                    nc.vector.tensor_copy(out=xm_sb[:, :], in_=xmt)
                    ctx["xm"] = xm_sb

                def s3():  # theta / phi
                    thp = tt[0:E, 89:114]
                    nc.tensor.matmul(
                        out=thp, lhsT=wtp[:, 0:E], rhs=ctx["xm"][:, :],
                        start=True, stop=True,
                    )
                    php = tt[0:E, 114:139]
                    nc.tensor.matmul(
                        out=php, lhsT=wtp[:, E : 2 * E],
                        rhs=ctx["xm"][:, :], start=True, stop=True,
                    )
                    th_sb = spool.tile([E, V], bf16, tag="th_sb")
                    nc.scalar.activation(
                        out=th_sb[:, :], in_=thp, func=ACTF.Identity,
                        bias=btpf[:, 0:1],
                    )
                    ph_sb = spool.tile([E, V], bf16, tag="ph_sb")
                    nc.vector.tensor_scalar(
                        out=ph_sb[:, :], in0=php,
                        scalar1=btpf[:, 1:2], scalar2=None, op0=ALU.add,
                    )
                    ctx["th"], ctx["ph"] = th_sb, ph_sb

                def s4():  # sim = theta^T @ phi
                    simp = tt[0:V, 139:164]
                    nc.tensor.matmul(
                        out=simp, lhsT=ctx["th"][:, :],
                        rhs=ctx["ph"][:, :], start=True, stop=True,
                    )
                    ctx["simp"] = simp

                def s5():  # softmax -> cmb (bf16); row sums fused into Exp
                    ex = spool.tile([V, V], f32, tag="ex")
                    rs = spool.tile([V, 1], f32, tag="rs")
                    nc.scalar.activation(
                        out=ex[:, :], in_=ctx["simp"], func=ACTF.Exp,
                        accum_out=rs[:, :],
                    )
                    rr = spool.tile([V, 1], f32, tag="rr")
                    nc.vector.reciprocal(out=rr[:, :], in_=rs[:, :])
                    cmb = spool.tile([V, V], bf16, tag="cm")
                    nc.gpsimd.tensor_scalar(
                        out=cmb[:, :], in0=ex[:, :],
                        scalar1=rr[:, 0:1], scalar2=None, op0=ALU.mult,
                    )
                    ctx["cmb"] = cmb

                def s6():  # blockdiag(cm) via 5 selector matmuls, then
                    # bd3 = bgab3 (static A+B blockdiags + bias row) + tiled
                    # blockdiag(cm) in one wide vector op.
                    bdcmp = tt[0:CH, 176:301]
                    for tau in range(5):
                        nc.tensor.matmul(
                            out=bdcmp[:, 25 * tau : 25 * tau + 25],
                            lhsT=selc[:, tau * CH : (tau + 1) * CH],
                            rhs=ctx["cmb"][:, :],
                            start=True, stop=True,
                        )
                    bd3 = spool.tile([CH, S * 125], bf16, tag="bd3")
                    nc.vector.tensor_tensor(
                        out=bd3[:, :].rearrange("p (s w) -> p s w", w=125),
                        in0=bgab3f.rearrange("p (s w) -> p s w", w=125),
                        in1=bdcmp[:, None, :].broadcast_to([CH, S, 125]),
                        op=ALU.add,
                    )
                    ctx["bd3"] = bd3

                return [s0, s1, s2, s3, s4, s5, s6]

            def phase_a_groups(ctx, weave=None):
                """Emit all 15 step1 groups, weaving tiny steps of the
                previous sample between the early groups."""
                weave = dict(weave or {})
                for g in range(15):
                    emit_group(ctx, g)
                    if g in weave:
                        weave[g]()

            def phase_b_state(n, fine_tail=False, flip=False):
                o_sb = opool.tile([125, NCH * Co], bf16, tag="o")
                # (group_size, copy_engine); engines: a=ACT, v=DVE, p=Pool
                if flip:
                    plan = [(8, "v"), (8, "a"), (8, "v"), (8, "a"), (8, "v"),
                            (8, "a"), (8, "v")]
                else:
                    plan = [(8, "a"), (8, "v"), (8, "a"), (8, "v"), (8, "a"),
                            (8, "v"), (8, "a")]
                plan += ([(2, "v"), (1, "a"), (1, "v")] if fine_tail
                         else [(4, "v")])
                dmas = {2: (0, 24), 5: (24, 48)}
                if fine_tail:
                    dmas.update({6: (48, 56), 9: (56, 60)})
                else:
                    dmas.update({7: (48, 60)})
                ch0s, c = [], 0
                for nch, _ in plan:
                    ch0s.append(c)
                    c += nch
                return {"n": n, "o": o_sb, "plan": plan, "dmas": dmas,
                        "ch0s": ch0s}

            def phase_b_group(bs, ctx, g):
                """step2 psum group g: s-accumulated matmuls + copy + DMA."""
                n, o_sb = bs["n"], bs["o"]
                y_sb, bd3 = ctx["y"], ctx["bd3"]
                nch, eng = bs["plan"][g]
                ch0 = bs["ch0s"][g]
                op = po.tile([CH, G2 * Co], f32, tag="p2")
                for s in range(S):
                    for j in range(nch):
                        ch = ch0 + j
                        nc.tensor.matmul(
                            out=op[0:125, j * Co : (j + 1) * Co],
                            lhsT=bd3[:, s * 125 : (s + 1) * 125],
                            rhs=y_sb[:, ch * W2 + s * Co : ch * W2 + (s + 1) * Co],
                            start=(s == 0 and j == 0),
                            stop=(s == S - 1 and j == nch - 1),
                        )
                dst = o_sb[:, ch0 * Co : (ch0 + nch) * Co]
                src = op[0:125, 0 : nch * Co]
                if eng == "a":
                    nc.scalar.copy(out=dst, in_=src)
                else:
                    nc.vector.tensor_copy(out=dst, in_=src)
                if g in bs["dmas"]:
                    lo, hi = bs["dmas"][g]
                    # the very last piece issues from the (idle) ACT queue so
                    # it doesn't serialize behind the previous SP issue
                    q = nc.scalar if g == 9 else nc.sync
                    q.dma_start(
                        out=out_ext[n][:, lo * Co : hi * Co],
                        in_=o_sb[:, lo * Co : hi * Co],
                    )

            # pipeline. Each round n: step1 groups of sample n, with
            # sample n's x^T matmuls early (g1-g6, x was prefetched last
            # round), its tiny chain at g7-g13, and the PREVIOUS sample's
            # step2 groups at every other position. This keeps the PE fed
            # while every psum->sbuf copy gets ~3 group-times of runway.
            ctxs = [phase_a_start(0)]
            consts_p2()
            st = None

            # round 0: xt woven 6 chunks/group for uniform PE pacing
            for g in range(15):
                if g >= 11 and st is None:
                    st = tiny_steps(ctxs[0])
                emit_group(ctxs[0], g, xt="none",
                           mid=st[g - 11] if g >= 11 else None)
                if 1 <= g <= 10:
                    emit_xt(ctxs[0], (g - 1) * 6, g * 6)
                if g == 8:
                    ctxs.append(phase_a_start(1))
            for k in (4, 5, 6):
                st[k]()
            for n in (1, 2):
                bs = phase_b_state(n - 1)
                st = None
                for g in range(15):
                    if g >= 7 and st is None:
                        st = tiny_steps(ctxs[n])
                    midf = None
                    if 7 <= g <= 13:
                        midf = st[g - 7]
                    emit_group(ctxs[n], g, xt="none", mid=midf)
                    if 1 <= g <= 6:
                        emit_xt(ctxs[n], (g - 1) * 10, g * 10)
                    if g == 8 and n == 1:
                        ctxs.append(phase_a_start(2))
                    if g == 0 and n == 2:
                        ctxs.append(phase_a_start(3))
                    if n == 2 and g >= 8:
                        # sample 3's x^T matmuls run here so its tiny chain
                        # can start at round 3 g0 and B3 can interleave
                        lo = (g - 8) * 9
                        emit_xt(ctxs[3], lo, min(lo + 9, NCH))
                    if g in (2, 4, 6, 8, 10, 12, 14):
                        phase_b_group(bs, ctxs[n - 1], g // 2 - 1)
                phase_b_group(bs, ctxs[n - 1], 7)
            # round 3: T3 at g0-6; B2 and B3's first groups share positions
            bs2 = phase_b_state(2)
            bs3 = phase_b_state(3, fine_tail=True)
            st3 = tiny_steps(ctxs[3])
            sched = {2: [(bs2, 2, 0)], 4: [(bs2, 2, 1)], 6: [(bs2, 2, 2)],
                     7: [(bs2, 2, 3)], 8: [(bs3, 3, 0)], 9: [(bs2, 2, 4)],
                     10: [(bs3, 3, 1)], 11: [(bs2, 2, 5)],
                     12: [(bs3, 3, 2), (bs2, 2, 6)],
                     13: [(bs3, 3, 3), (bs2, 2, 7)],
                     14: [(bs3, 3, 4), (bs3, 3, 5), (bs3, 3, 6)]}
            for g in range(15):
                midf = st3[g] if g <= 6 else None
                emit_group(ctxs[3], g, xt="none", mid=midf)
                for bsx, cn, j in sched.get(g, []):
                    phase_b_group(bsx, ctxs[cn], j)
            for j in range(7, len(bs3["plan"])):
                phase_b_group(bs3, ctxs[3], j)

    nc.finalize()
    return nc


def _prep_consts(A, B, W_theta, b_theta, W_phi, b_phi, W_big, b_big):
    f = np.float32
    ct = np.zeros((CH, CF), dtype=f)  # filled in f32, cast to bf16 at return
    ct[0:C, 0:576] = (
        W_big.reshape(S, Co, S, C).transpose(3, 0, 2, 1).reshape(C, 3 * S * Co)
    )
    ct[0:C, 576:640] = np.eye(C, dtype=f)
    ct[0:1, 640:832] = b_big.reshape(1, S * Co)
    ct[0:C, 832:960] = np.concatenate([W_theta.T, W_phi.T], axis=1)
    ct[0:E, 960] = b_theta
    ct[0:E, 961] = b_phi
    ct[0:125, 962:987] = np.tile(np.eye(V, dtype=f), (5, 1))
    # selector lhsTs: sel_tau[v, p] = 1 iff p == 25*tau + v
    for tau in range(5):
        for v in range(V):
            ct[v, 987 + tau * CH + 25 * tau + v] = 1.0
    # bd3 background: blockdiag(A_s+B_s) per s-block + bias row in s=0
    AB = (A + B).astype(f)
    for sb in range(S):
        for tau in range(5):
            r0, c0 = 25 * tau, 1617 + 125 * sb + 25 * tau
            ct[r0 : r0 + 25, c0 : c0 + 25] = AB[sb]
    ct[125, 1617:1742] = 1.0
    return {"consts": ct.astype(ml_dtypes.bfloat16)}


def _prep_x(x):
    bf = ml_dtypes.bfloat16
    xp = np.zeros((N, CP, NCH, CH), dtype=bf)
    xp[:, :C, :, :125] = x.reshape(N, C, NCH, 125).astype(bf)
    xp[:, C, :, 125] = 1.0  # bias column per chunk
    return xp.reshape(N, CP, XW)


def kernel(x, A, B, W_theta, b_theta, W_phi, b_phi, W_big, b_big, _profile=None):
    _import_concourse()
    from concourse.bass_utils import run_bass_kernel_spmd

    x = np.asarray(x, dtype=np.float32)
    xp = _prep_x(x)

    consts = _prep_consts(
        np.asarray(A, np.float32), np.asarray(B, np.float32),
        np.asarray(W_theta, np.float32), np.asarray(b_theta, np.float32),
        np.asarray(W_phi, np.float32), np.asarray(b_phi, np.float32),
        np.asarray(W_big, np.float32), np.asarray(b_big, np.float32),
    )

    if "nc" not in _CACHE:
        _CACHE["nc"] = _build_nc()
    nc = _CACHE["nc"]

    in_maps = []
    for i in range(NCORES):
        m = {"x": np.ascontiguousarray(xp[i * NL : (i + 1) * NL])}
        m.update(consts)
        in_maps.append(m)

    kw = {}
    if _profile:
        kw = dict(trace=True, tmpdir=_profile)
    res = run_bass_kernel_spmd(nc, in_maps, list(range(NCORES)), **kw)

    out = np.empty((N, Co, T, V), dtype=np.float32)
    for i in range(NCORES):
        buf = np.asarray(res.results[i]["out"], dtype=np.float32).reshape(
            NL, 5, V, NCH, Co
        )
        # [n, tau, w, ch, co] -> [n, co, ch, tau, w]
        out[i * NL : (i + 1) * NL] = (
            buf.transpose(0, 4, 3, 1, 2).reshape(NL, Co, T, V)
        )
    if _profile:
        _CACHE["exec_time_ns"] = res.exec_time_ns
    return out



# revision 62
# speedup vs baseline: 1.1026x; 1.1026x over previous
"""AdaptiveGraphConv Trainium2 kernel — 8-core batch-parallel Bass/Tile.

Math (per sample n):
  xm     = mean_t x[n]                                  [C, V]
  sim    = (W_th xm + b_th)^T (W_ph xm + b_ph)          [V, V]
  Cmat   = softmax_w(sim)
  adap_s = A[s] + B[s] + Cmat                           [V, V]
  out[n] = sum_s W_eff_s @ x[n] @_v adap_s + b_eff      [Co, T, V]
where W_eff_s[co,c] = sum_sg W_big[sg*Co+co, s*C+c], b_eff = sum_sg b_big[sg*Co:+Co].

Softmax is invariant to per-row(v) offsets, so
  sim ~ xm^T M xm + 1 (b_th^T W_ph xm)   with M = W_th^T W_ph
and the v-only/constant terms are dropped. Host sends Mv = [M^T | W_ph^T b_th].

Device dataflow (per core, 4 samples). T*V splits into 60 chunks of
(5t, 25v) = 125 elements; chunk PAIRS share free columns with chunk 2j in
partitions 0-63 and 2j+1 in 64-127 ("half" packing):
  step1: matmul(lhsT = x half [64, 125], rhs = wstack half [64, 192])
         -> y chunk [(5t,25v), (s,co)] in PSUM, groups of 4 chunks
         -> y_sb bf16 rows 0:125 (ACT/DVE copies); y_sb row 125 is the
         constant b_eff row (s=0 block), DMA-prefilled once per buffer.
  mean : matmul(lhsT = x pair [128, 125], rhs = [I64;I64]) accumulated over
         30 pairs -> xtp [125, 64] (exact, K=128)
  tiny : xm = xta^T obd (obd carries 1/T); t1 = [M^T|v]^T xm;
         sim = [xm;1]^T [t1;r]; softmax -> cm; blockdiag -> bd3 (bias row
         from static A+B background consts)
  step2: matmul(lhsT = bd3 s-block [126,125], rhs = y chunk [126, 64co])
         accumulated over s -> [125, 64co] -> o_sb bf16 -> DMA (host upcasts)
"""

import numpy as np
import ml_dtypes

N, C, T, V, S, E, Co = 32, 64, 300, 25, 3, 64, 64
NCORES = 8
NL = N // NCORES          # samples per core = 4
CH = 126                  # bd3 partitions: 125 data + bias row
NCH = 60                  # chunks per sample (T/5)
NPR = 30                  # chunk pairs per sample
W2 = S * Co               # 192 = y columns per chunk
XW = NPR * 125            # 3750 = x free size per sample (pair-packed)
G1 = 4                    # step1 chunks per psum group (15 groups)
G2 = 8                    # step2 chunks per psum group
CF = 1352                 # packed consts free size (even cols: 4B row stride)
YW = NCH * W2             # 11520 = y_sb free size
N_WARM = 32               # PE warmup matmuls (bridge DMA cold start)

# schedule tuning knobs (see _build_nc); sweepable via kernel.CFG.update()
# bslots12: R1/R2 slot plan — (g, 0) -> next B(n-1) group, (g, 1) -> next
# B(n) group. r3plan: (g, 2) -> next B2 group, (g, 3) -> next B3 group.
CFG = {
    "nwarm": 26,
    "split0": (),         # R0 groups with split copies
    "split12": (),        # R1/R2 groups with split copies
    "split3": (),         # R3 groups with split copies
    "xt1_start": 9,       # R0 weave start for xt(1)
    "r0xt6": False,       # xt(0) at 6/group g1-5 (else 5/group g1-6)
    "tiny0_start": 8,     # R0 tiny chain start group
    "tiny12_start": 2,    # R1/R2 tiny chain start group
    "ceng0": "avavavaaaavavav",    # R0 step1 copy engines per group
    "ceng12": "avavavavavaavva",   # R1/R2 step1 copy engines
    "ceng3": "avavavavavvaava",    # R3 step1 copy engines
    "beng": "avavavavav",          # step2 copy engines per group idx
    "r0bslots": (13, 13, 14),   # R0 slots for B0 j0..k-1
    "bslots12": ((2, 0), (4, 0), (4, 0), (5, 0), (7, 0),
                 (10, 1), (12, 1), (13, 1)),
    "r3plan": ((1, 2), (4, 2), (4, 2), (5, 2), (5, 2), (6, 3),
               (7, 3), (10, 3), (11, 3), (13, 3), (14, 3)),
}

_YPERM = (0, 2, 1, 3)  # stored slot of chunk ch within its group


def _ycol(ch):
    """y_sb column of chunk ch (chunks stored group-interleaved)."""
    return ((ch >> 2) * 4 + _YPERM[ch & 3]) * W2


_CACHE = {}


def _import_concourse():
    try:
        import concourse  # noqa: F401
    except ImportError:
        import sys

        for p in ("/opt/trn_rl_repo", "/root/.axon_site/_ro/trn_rl_repo"):
            if p not in sys.path:
                sys.path.insert(0, p)


def _build_nc():
    _import_concourse()
    import concourse.bass as bass
    import concourse.bacc as bacc
    import concourse.mybir as mybir
    from concourse import tile

    dt = mybir.dt
    f32, bf16 = dt.float32, dt.bfloat16
    ALU = mybir.AluOpType
    ACTF = mybir.ActivationFunctionType

    nc = bacc.Bacc(None, target_bir_lowering=False)

    x_ext = nc.declare_dram_parameter("x", [NL, 128, XW], bf16, isOutput=False)
    c_ext = nc.declare_dram_parameter("consts", [128, CF], bf16, isOutput=False)
    yr_ext = nc.declare_dram_parameter("yrow", [1, YW], bf16, isOutput=False)
    out_ext = nc.declare_dram_parameter(
        "out", [NL, 125, NCH * Co], bf16, isOutput=True
    )

    with tile.TileContext(nc) as tc:
        with (
            tc.tile_pool(name="const", bufs=1) as cpool,
            tc.tile_pool(name="xin", bufs=2) as xpool,
            tc.tile_pool(name="y", bufs=2) as ypool,
            tc.tile_pool(name="osb", bufs=2) as opool,
            tc.tile_pool(name="small", bufs=2) as spool,
            tc.tile_pool(name="xmaug", bufs=2) as xmpool,
            tc.tile_pool(name="p1", bufs=2, space="PSUM") as pq,
            tc.tile_pool(name="p2", bufs=2, space="PSUM") as po,
            tc.tile_pool(name="pxs", bufs=1, space="PSUM") as pxs,
            tc.tile_pool(name="ptiny", bufs=1, space="PSUM") as pt,
        ):
            # ---------------- PE warmup ----------------
            # tiny matmuls bridge the DMA startup so the PE is continuously
            # busy into its 3us ramp when real work arrives.
            wz = cpool.tile([1, Co], bf16)
            nc.gpsimd.memset(wz[:, :], 0.0)
            n_warm = CFG["nwarm"]
            # tt: single psum bank shared by warmup and all tiny outs;
            # xtp separate (its accumulation group stays open all round and
            # a start=True in the same bank would pending-zero it).
            tt = pt.tile([CH, 264], f32, tag="tt")
            xtq = pxs.tile([125, Co], f32, tag="xt")
            warm = tt[0:1, 0:Co]
            xtp = xtq[:, :]
            xm_p = tt[0:64, 64:89]
            t1p = tt[0:65, 89:114]
            simp = tt[0:25, 114:139]
            bdcmp = tt[0:CH, 139:264]
            for _ in range(n_warm):
                nc.tensor.matmul(
                    out=warm, lhsT=wz[:, 0:1], rhs=wz[:, :],
                    start=True, stop=True,
                )

            # ---------------- constants ----------------
            ct = cpool.tile([128, CF], bf16)
            # p1: wstack only (feeds the first step1 group) — smallest first
            nc.sync.dma_start(out=ct[:, 0:256], in_=c_ext[:, 0:256])

            wstack = ct[:, 0:256]           # [W_eff cat | I64] x2 halves
            selc = ct[0:V, 256:886]         # 5 tau-selectors [25, 126]
            bgab3f = ct[0:CH, 886:1261]     # A+B blockdiag bg + bias row
            mvcat = ct[0:C, 1261:1326]      # [M^T | W_ph^T b_th]
            obd = ct[0:125, 1326:1351]      # tau-sum selector * (1/T)

            # y_sb double buffer; row 125 = const b_eff row via DMA, filled
            # once per buffer and never overwritten (copies write 0:125).
            y_bufs = [
                ypool.tile([CH, YW], bf16, tag="y", name=f"ybuf{i}")
                for i in range(2)
            ]

            # xm_aug buffers; row 64 = ones (memset once per buffer)
            xm_bufs = [
                xmpool.tile([65, V], bf16, tag="xm", name=f"xmbuf{i}")
                for i in range(2)
            ]

            def consts_p2():
                nc.gpsimd.dma_start(out=ct[:, 256:CF], in_=c_ext[:, 256:CF])
                for xb in xm_bufs:
                    nc.gpsimd.memset(xb[64:65, :], 1.0)

            def fill_yrow(i, qa, qb):
                """Fill y buffer i's bias row in two half-DMAs (the cost
                model charges per-partition bytes, so this single-partition
                row is expensive — split across two queues)."""
                yb = y_bufs[i]
                h = YW // 2
                qa.dma_start(out=yb[125:126, 0:h], in_=yr_ext[:, 0:h])
                qb.dma_start(out=yb[125:126, h:YW], in_=yr_ext[:, h:YW])

            # ---------------- per-sample phases ----------------
            def phase_a_start(n):
                """Allocate tiles + x DMA for sample n."""
                x_sb = xpool.tile([128, XW], bf16, tag="x")
                if n == 0:
                    # piece boundaries must be 4-byte aligned (even bf16 cols)
                    cuts = [0, 250, 876, 1876, 2812, XW]
                    qs = [nc.sync, nc.gpsimd, nc.sync, nc.gpsimd, nc.sync]
                else:
                    cuts = [0, 1876, XW]
                    qs = [nc.sync, nc.gpsimd]
                for q, (lo, hi) in zip(qs, zip(cuts, cuts[1:])):
                    q.dma_start(out=x_sb[:, lo:hi], in_=x_ext[n][:, lo:hi])
                return {"x": x_sb, "y": y_bufs[n % 2], "n": n}

            def emit_xt(ctx, lo, hi):
                """x-sum accumulation matmuls for pairs [lo, hi), K=128."""
                x_sb = ctx["x"]
                for pr in range(lo, hi):
                    nc.tensor.matmul(
                        out=xtp,
                        lhsT=x_sb[:, pr * 125 : (pr + 1) * 125],
                        rhs=wstack[:, 192:256],
                        start=(pr == 0),
                        stop=(pr == NPR - 1),
                    )

            def emit_group(ctx, g, mid=None, split=False, ceng="a"):
                """step1 psum group g: 4 y-matmuls, then psum->sbuf copy.
                A matmul out may not cross a 512-f32 psum bank boundary, so
                chunk j sits at column (j//2)*512 + (j%2)*192.
                split=True copies the two banks on BOTH engines in parallel
                (halves the copy latency when PE is pq-recycle-bound)."""
                x_sb, y_sb = ctx["x"], ctx["y"]
                yp = pq.tile([125, 1024], f32, tag="p1")
                # chunk order (4g, 4g+2, 4g+1, 4g+3): each psum bank's
                # accumulation group then uses a single partition half (a
                # group spanning two tile_positions breaks the HW path).
                # y_sb stores chunks in this interleaved slot order; step2
                # remaps columns via _ycol.
                for j in range(G1):
                    ch = G1 * g + (0, 2, 1, 3)[j]
                    pr, half = ch >> 1, ch & 1
                    col = (j // 2) * 512 + (j % 2) * W2
                    nc.tensor.matmul(
                        out=yp[:, col : col + W2],
                        lhsT=x_sb[64 * half : 64 * half + 64,
                                  pr * 125 : (pr + 1) * 125],
                        rhs=wstack[64 * half : 64 * half + 64, 0:W2],
                        start=(j % 2 == 0),
                        stop=(j % 2 == 1),
                    )
                if mid is not None:
                    mid()  # latency-critical ops enqueue ahead of the copy
                c0 = g * G1 * W2
                # GPSIMD cannot access PSUM, so psum->sbuf copies are split
                # across ACT and DVE only.
                if split:
                    ea, eb = ((nc.scalar.copy, nc.vector.tensor_copy)
                              if ceng == "a" else
                              (nc.vector.tensor_copy, nc.scalar.copy))
                    ea(out=y_sb[0:125, c0 : c0 + 2 * W2],
                       in_=yp[:, 0 : 2 * W2])
                    eb(out=y_sb[0:125, c0 + 2 * W2 : c0 + 4 * W2],
                       in_=yp[:, 512 : 512 + 2 * W2])
                    return
                dst = y_sb[
                    0:125, c0 : c0 + G1 * W2
                ].rearrange("p (b w) -> p b w", w=2 * W2)
                src = yp[:, :].rearrange("p (b w) -> p b w", w=512)[
                    :, :, 0 : 2 * W2
                ]
                if ceng == "a":
                    nc.scalar.copy(out=dst, in_=src)
                else:
                    nc.vector.tensor_copy(out=dst, in_=src)

            def tiny_steps(ctx):
                """Mean/softmax/bd3 chain as 5 steps; weave each between PE
                group emissions so cross-engine hops never stall the PE."""
                n = ctx["n"]
                xm_aug = xm_bufs[n % 2]

                def s0():  # xta copy (DVE)
                    xta = spool.tile([125, C], bf16, tag="xta")
                    nc.vector.tensor_copy(out=xta[:, :], in_=xtp)
                    ctx["xta"] = xta

                def s1():  # xm = xta^T obd  (obd carries 1/T)
                    nc.tensor.matmul(
                        out=xm_p, lhsT=ctx["xta"][:, :], rhs=obd,
                        start=True, stop=True,
                    )
                    nc.vector.tensor_copy(out=xm_aug[0:C, :], in_=xm_p)

                def s2():  # t1 = [M^T | v]^T xm
                    nc.tensor.matmul(
                        out=t1p, lhsT=mvcat, rhs=xm_aug[0:C, :],
                        start=True, stop=True,
                    )
                    t1sb = spool.tile([65, V], bf16, tag="t1")
                    nc.scalar.copy(out=t1sb[:, :], in_=t1p)
                    ctx["t1"] = t1sb

                def s3():  # sim = [xm;1]^T [t1;r], then softmax -> cm
                    nc.tensor.matmul(
                        out=simp, lhsT=xm_aug[:, :], rhs=ctx["t1"][:, :],
                        start=True, stop=True,
                    )
                    ex = spool.tile([V, V], f32, tag="ex")
                    rs = spool.tile([V, 1], f32, tag="rs")
                    nc.scalar.activation(
                        out=ex[:, :], in_=simp, func=ACTF.Exp,
                        accum_out=rs[:, :],
                    )
                    rr = spool.tile([V, 1], f32, tag="rr")
                    nc.vector.reciprocal(out=rr[:, :], in_=rs[:, :])
                    cmb = spool.tile([V, V], bf16, tag="cm")
                    nc.vector.tensor_scalar(
                        out=cmb[:, :], in0=ex[:, :],
                        scalar1=rr[:, 0:1], scalar2=None, op0=ALU.mult,
                    )
                    ctx["cmb"] = cmb

                def s4():  # blockdiag(cm) via 5 selector matmuls, then
                    # bd3 = bgab3 (static A+B blockdiags + bias row) + tiled
                    # blockdiag(cm) in one wide vector op.
                    for tau in range(5):
                        nc.tensor.matmul(
                            out=bdcmp[:, 25 * tau : 25 * tau + 25],
                            lhsT=selc[:, tau * CH : (tau + 1) * CH],
                            rhs=ctx["cmb"][:, :],
                            start=True, stop=True,
                        )
                    bd3 = spool.tile([CH, S * 125], bf16, tag="bd3")
                    nc.vector.tensor_tensor(
                        out=bd3[:, :].rearrange("p (s w) -> p s w", w=125),
                        in0=bgab3f.rearrange("p (s w) -> p s w", w=125),
                        in1=bdcmp[:, None, :].broadcast_to([CH, S, 125]),
                        op=ALU.add,
                    )
                    ctx["bd3"] = bd3

                return [s0, s1, s2, s3, s4]

            def phase_b_state(n, fine_tail=False, flip=False):
                o_sb = opool.tile([125, NCH * Co], bf16, tag="o")
                # (group_size, copy_engine); engines from CFG["beng"]
                be = CFG["beng"]
                sizes = ([8] * 7 + [2, 1, 1]) if fine_tail else [8] * 7 + [4]
                plan = [(s, be[i % len(be)]) for i, s in enumerate(sizes)]
                dmas = {2: (0, 24), 5: (24, 48)}
                if fine_tail:
                    dmas.update({6: (48, 56), 7: (56, 58), 8: (58, 59),
                                 9: (59, 60)})
                else:
                    dmas.update({7: (48, 60)})
                ch0s, c = [], 0
                for nch, _ in plan:
                    ch0s.append(c)
                    c += nch
                return {"n": n, "o": o_sb, "plan": plan, "dmas": dmas,
                        "ch0s": ch0s}

            def phase_b_group(bs, ctx, g):
                """step2 psum group g: s-accumulated matmuls + copy + DMA."""
                n, o_sb = bs["n"], bs["o"]
                y_sb, bd3 = ctx["y"], ctx["bd3"]
                nch, eng = bs["plan"][g]
                ch0 = bs["ch0s"][g]
                op = po.tile([125, G2 * Co], f32, tag="p2")
                for s in range(S):
                    for j in range(nch):
                        ch = ch0 + j
                        nc.tensor.matmul(
                            out=op[:, j * Co : (j + 1) * Co],
                            lhsT=bd3[:, s * 125 : (s + 1) * 125],
                            rhs=y_sb[:, _ycol(ch) + s * Co
                                     : _ycol(ch) + (s + 1) * Co],
                            start=(s == 0 and j == 0),
                            stop=(s == S - 1 and j == nch - 1),
                        )
                dst = o_sb[:, ch0 * Co : (ch0 + nch) * Co]
                src = op[:, 0 : nch * Co]
                if eng == "a":
                    nc.scalar.copy(out=dst, in_=src)
                else:
                    nc.vector.tensor_copy(out=dst, in_=src)
                if g in bs["dmas"]:
                    lo, hi = bs["dmas"][g]
                    q = {6: nc.gpsimd, 7: nc.sync, 8: nc.gpsimd,
                         9: nc.scalar}.get(g, nc.sync)
                    q.dma_start(
                        out=out_ext[n][:, lo * Co : hi * Co],
                        in_=o_sb[:, lo * Co : hi * Co],
                    )

            # pipeline. Round n emits sample n's 15 step1 groups; sample n's
            # x-sums run one round EARLY (n>=1) so its tiny chain can start
            # at round n g0 and B(n) can begin mid-round, chasing the y
            # copies. B(n) groups left over drain into round n+1's slots.
            ctxs = [phase_a_start(0)]
            ctxs.append(phase_a_start(1))  # x1 up front (buffer B)
            consts_p2()
            if "no_yrow" not in CFG.get("flags", ()):
                fill_yrow(0, nc.sync, nc.gpsimd)

            # round 0: xt(0) at g1-6 paced by x arrival; tiny(0) at g7-11;
            # xt(1) at g9-14; B(0) j0-1 at g13-14.
            bs0 = phase_b_state(0)
            st = tiny_steps(ctxs[0])
            xs1 = CFG["xt1_start"]
            t0s = CFG["tiny0_start"]
            r0b = list(CFG["r0bslots"])
            FL = CFG.get("flags", ())
            for g in range(15):
                midf = st[g - t0s] if t0s <= g <= t0s + 4 else None
                if "no_tiny" in FL:
                    midf = None
                emit_group(ctxs[0], g, mid=midf, split=(g in CFG["split0"]),
                           ceng=CFG["ceng0"][g])
                if "no_xt" not in FL:
                    if CFG["r0xt6"]:
                        if 1 <= g <= 5:
                            emit_xt(ctxs[0], (g - 1) * 6, g * 6)
                    elif 1 <= g <= 6:
                        emit_xt(ctxs[0], (g - 1) * 5, g * 5)
                    if "no_tiny" not in FL and xs1 <= g <= xs1 + 5:
                        emit_xt(ctxs[1], (g - xs1) * 5, (g - xs1 + 1) * 5)
                if "no_b0" in FL:
                    continue
                while r0b and r0b[0] == g:
                    r0b.pop(0)
                    phase_b_group(bs0, ctxs[0],
                                  len(CFG["r0bslots"]) - len(r0b) - 1)

            # rounds 1-2: tiny(n) at g0-4, B(n-1) remainder at odd g,
            # B(n) j0-2 at g12-14, xt(n+1) at g9-14.
            trunc = CFG.get("trunc", 4)
            bprev = bs0
            for n in (1, 2):
                if n >= trunc:
                    break
                bs = phase_b_state(n, flip=True)
                st = tiny_steps(ctxs[n])
                ctxs.append(phase_a_start(n + 1))
                fill_yrow(1, nc.sync, nc.gpsimd) if n == 1 else None
                nb_prev = (len(CFG["r0bslots"]) if n == 1 else
                           sum(1 for _, w in CFG["bslots12"] if w == 1))
                jnext = [nb_prev, 0]  # next group idx for [B(n-1), B(n)]
                slot = {}
                for g, who in CFG["bslots12"]:
                    slot.setdefault(g, []).append(who)
                t12 = CFG["tiny12_start"]
                for g in range(15):
                    midf = st[g - t12] if t12 <= g <= t12 + 4 else None
                    emit_group(ctxs[n], g, mid=midf,
                               split=(g in CFG["split12"]),
                               ceng=CFG["ceng12"][g])
                    for who in slot.get(g, []):
                        if who == 0:
                            phase_b_group(bprev, ctxs[n - 1], jnext[0])
                            jnext[0] += 1
                        else:
                            phase_b_group(bs, ctxs[n], jnext[1])
                            jnext[1] += 1
                    if 9 <= g <= 14:
                        emit_xt(ctxs[n + 1], (g - 9) * 5, (g - 8) * 5)
                bprev = bs
            # round 3: tiny(3) at g0-4; B(2) remainder at even g; B(3) from
            # g5 chasing its own y copies; fine tail drains after the loop.
            if trunc >= 4:
                bs3 = phase_b_state(3, fine_tail=True)
                st3 = tiny_steps(ctxs[3])
                jnext3 = [sum(1 for _, w in CFG["bslots12"] if w == 1), 0]
                slot3 = {}
                for g, who in CFG["r3plan"]:
                    slot3.setdefault(g, []).append(who)
                for g in range(15):
                    midf = st3[g] if g <= 4 else None
                    emit_group(ctxs[3], g, mid=midf,
                               split=(g in CFG["split3"]),
                               ceng=CFG["ceng3"][g])
                    for who in slot3.get(g, []):
                        if who == 2:
                            phase_b_group(bprev, ctxs[2], jnext3[0])
                            jnext3[0] += 1
                        else:
                            phase_b_group(bs3, ctxs[3], jnext3[1])
                            jnext3[1] += 1
                for j in range(jnext3[1], len(bs3["plan"])):
                    phase_b_group(bs3, ctxs[3], j)

    nc.finalize()
    return nc


def _prep_consts(A, B, W_theta, b_theta, W_phi, b_phi, W_big, b_big):
    f = np.float32
    ct = np.zeros((128, CF), dtype=f)
    # wstack: [W_eff cat over s | I64], duplicated in both partition halves
    weff = W_big.reshape(S, Co, S, C).sum(axis=0)  # [co, s, c]
    wst = np.zeros((64, 256), dtype=f)
    wst[:, 0:192] = weff.transpose(2, 1, 0).reshape(C, S * Co)
    wst[:, 192:256] = np.eye(C, dtype=f)
    ct[0:64, 0:256] = wst
    ct[64:128, 0:256] = wst
    # selector lhsTs: sel_tau[v, p] = 1 iff p == 25*tau + v
    for tau in range(5):
        for v in range(V):
            ct[v, 256 + tau * CH + 25 * tau + v] = 1.0
    # bd3 background: blockdiag(A_s+B_s) per s-block + bias row in s=0
    AB = (A + B).astype(f)
    for sb in range(S):
        for tau in range(5):
            r0, c0 = 25 * tau, 886 + 125 * sb + 25 * tau
            ct[r0 : r0 + 25, c0 : c0 + 25] = AB[sb]
    ct[125, 886:1011] = 1.0
    # Mv_cat = [ (W_th^T W_ph)^T | W_ph^T b_th ] = [ W_ph^T W_th | W_ph^T b_th ]
    ct[0:C, 1261:1325] = W_phi.T @ W_theta
    ct[0:C, 1325] = W_phi.T @ b_theta
    # obd: tau-sum selector with 1/T folded in
    ct[0:125, 1326:1351] = np.tile(np.eye(V, dtype=f), (5, 1)) / T

    b_eff = b_big.reshape(S, Co).sum(axis=0)
    yrow = np.zeros((NCH, W2), dtype=f)
    yrow[:, 0:Co] = b_eff
    return {
        "consts": ct.astype(ml_dtypes.bfloat16),
        "yrow": yrow.reshape(1, YW).astype(ml_dtypes.bfloat16),
    }


def _prep_x(x):
    bf = ml_dtypes.bfloat16
    # pair packing: chunk 2j -> partitions 0:64, chunk 2j+1 -> 64:128
    xv = x.reshape(N, C, NPR, 2, 125)
    xp = xv.transpose(0, 3, 1, 2, 4).reshape(N, 128, XW).astype(bf)
    return xp


def kernel(x, A, B, W_theta, b_theta, W_phi, b_phi, W_big, b_big, _profile=None):
    _import_concourse()
    from concourse.bass_utils import run_bass_kernel_spmd

    x = np.asarray(x, dtype=np.float32)
    xp = _prep_x(x)

    consts = _prep_consts(
        np.asarray(A, np.float32), np.asarray(B, np.float32),
        np.asarray(W_theta, np.float32), np.asarray(b_theta, np.float32),
        np.asarray(W_phi, np.float32), np.asarray(b_phi, np.float32),
        np.asarray(W_big, np.float32), np.asarray(b_big, np.float32),
    )

    if "nc" not in _CACHE:
        _CACHE["nc"] = _build_nc()
    nc = _CACHE["nc"]

    in_maps = []
    for i in range(NCORES):
        m = {"x": np.ascontiguousarray(xp[i * NL : (i + 1) * NL])}
        m.update(consts)
        in_maps.append(m)

    kw = {}
    if _profile:
        kw = dict(trace=True, tmpdir=_profile)
    res = run_bass_kernel_spmd(nc, in_maps, list(range(NCORES)), **kw)

    out = np.empty((N, Co, T, V), dtype=np.float32)
    for i in range(NCORES):
        buf = np.asarray(res.results[i]["out"], dtype=np.float32).reshape(
            NL, 5, V, NCH, Co
        )
        # [n, tau, w, ch, co] -> [n, co, ch, tau, w]
        out[i * NL : (i + 1) * NL] = (
            buf.transpose(0, 4, 3, 1, 2).reshape(NL, Co, T, V)
        )
    if _profile:
        _CACHE["exec_time_ns"] = res.exec_time_ns
    return out


# revision 63
# speedup vs baseline: 1.1117x; 1.0083x over previous
"""AdaptiveGraphConv Trainium2 kernel — 8-core batch-parallel Bass/Tile.

Math (per sample n):
  xm     = mean_t x[n]                                  [C, V]
  sim    = (W_th xm + b_th)^T (W_ph xm + b_ph)          [V, V]
  Cmat   = softmax_w(sim)
  adap_s = A[s] + B[s] + Cmat                           [V, V]
  out[n] = sum_s W_eff_s @ x[n] @_v adap_s + b_eff      [Co, T, V]
where W_eff_s[co,c] = sum_sg W_big[sg*Co+co, s*C+c], b_eff = sum_sg b_big[sg*Co:+Co].

Softmax is invariant to per-row(v) offsets, so
  sim ~ xm^T M xm + 1 (b_th^T W_ph xm)   with M = W_th^T W_ph
and the v-only/constant terms are dropped. Host sends Mv = [M^T | W_ph^T b_th].

Device dataflow (per core, 4 samples). T*V splits into 60 chunks of
(5t, 25v) = 125 elements; chunk PAIRS share free columns with chunk 2j in
partitions 0-63 and 2j+1 in 64-127 ("half" packing):
  step1: matmul(lhsT = x half [64, 125], rhs = wstack half [64, 192])
         -> y chunk [(5t,25v), (s,co)] in PSUM, groups of 4 chunks
         -> y_sb bf16 rows 0:125 (ACT/DVE copies); y_sb row 125 is the
         constant b_eff row (s=0 block), DMA-prefilled once per buffer.
  mean : matmul(lhsT = x pair [128, 125], rhs = [I64;I64]) accumulated over
         30 pairs -> xtp [125, 64] (exact, K=128)
  tiny : xm = xta^T obd (obd carries 1/T); t1 = [M^T|v]^T xm;
         sim = [xm;1]^T [t1;r]; softmax -> cm; blockdiag -> bd3 (bias row
         from static A+B background consts)
  step2: matmul(lhsT = bd3 s-block [126,125], rhs = y chunk [126, 64co])
         accumulated over s -> [125, 64co] -> o_sb bf16 -> DMA (host upcasts)
"""

import numpy as np
import ml_dtypes

N, C, T, V, S, E, Co = 32, 64, 300, 25, 3, 64, 64
NCORES = 8
NL = N // NCORES          # samples per core = 4
CH = 126                  # bd3 partitions: 125 data + bias row
NCH = 60                  # chunks per sample (T/5)
NPR = 30                  # chunk pairs per sample
W2 = S * Co               # 192 = y columns per chunk
XW = NPR * 125            # 3750 = x free size per sample (pair-packed)
G1 = 4                    # step1 chunks per psum group (15 groups)
G2 = 8                    # step2 chunks per psum group
CF = 1352                 # packed consts free size (even cols: 4B row stride)
YW = NCH * W2             # 11520 = y_sb free size
N_WARM = 32               # PE warmup matmuls (bridge DMA cold start)

# schedule tuning knobs (see _build_nc); sweepable via kernel.CFG.update()
# bslots12: R1/R2 slot plan — (g, 0) -> next B(n-1) group, (g, 1) -> next
# B(n) group. r3plan: (g, 2) -> next B2 group, (g, 3) -> next B3 group.
CFG = {
    "nwarm": 26,
    "split0": (),         # R0 groups with split copies
    "split12": (),        # R1/R2 groups with split copies
    "split3": (),         # R3 groups with split copies
    "xt1_start": 8,       # R0 weave start for xt(1)
    "r0xt6": False,       # xt(0) at 6/group g1-5 (else 5/group g1-6)
    "tiny0_start": 8,     # R0 tiny chain start group
    "tiny12_start": 1,    # R1/R2 tiny chain start group
    "ceng0": "avavavaaavvavav",    # R0 step1 copy engines per group
    "ceng12": "avavvaavavaavva",   # R1/R2 step1 copy engines
    "ceng3": "avavavavavvaava",    # R3 step1 copy engines
    "beng": "avavavavaa",          # step2 copy engines per group idx
    "r0bslots": (12, 13, 14),   # R0 slots for B0 j0..k-1
    "bslots12": ((2, 0), (3, 0), (5, 0), (7, 0), (7, 0),
                 (9, 1), (10, 1), (12, 1)),
    "r3plan": ((3, 2), (5, 2), (5, 2), (5, 2), (6, 2), (6, 3),
               (6, 3), (10, 3), (12, 3), (12, 3), (14, 3)),
}

_YPERM = (0, 2, 1, 3)  # stored slot of chunk ch within its group


def _ycol(ch):
    """y_sb column of chunk ch (chunks stored group-interleaved)."""
    return ((ch >> 2) * 4 + _YPERM[ch & 3]) * W2


_CACHE = {}


def _import_concourse():
    try:
        import concourse  # noqa: F401
    except ImportError:
        import sys

        for p in ("/opt/trn_rl_repo", "/root/.axon_site/_ro/trn_rl_repo"):
            if p not in sys.path:
                sys.path.insert(0, p)


def _build_nc():
    _import_concourse()
    import concourse.bass as bass
    import concourse.bacc as bacc
    import concourse.mybir as mybir
    from concourse import tile

    dt = mybir.dt
    f32, bf16 = dt.float32, dt.bfloat16
    ALU = mybir.AluOpType
    ACTF = mybir.ActivationFunctionType

    nc = bacc.Bacc(None, target_bir_lowering=False)

    x_ext = nc.declare_dram_parameter("x", [NL, 128, XW], bf16, isOutput=False)
    c_ext = nc.declare_dram_parameter("consts", [128, CF], bf16, isOutput=False)
    yr_ext = nc.declare_dram_parameter("yrow", [1, YW], bf16, isOutput=False)
    out_ext = nc.declare_dram_parameter(
        "out", [NL, 125, NCH * Co], bf16, isOutput=True
    )

    with tile.TileContext(nc) as tc:
        with (
            tc.tile_pool(name="const", bufs=1) as cpool,
            tc.tile_pool(name="xin", bufs=2) as xpool,
            tc.tile_pool(name="y", bufs=2) as ypool,
            tc.tile_pool(name="osb", bufs=2) as opool,
            tc.tile_pool(name="small", bufs=2) as spool,
            tc.tile_pool(name="xmaug", bufs=2) as xmpool,
            tc.tile_pool(name="p1", bufs=2, space="PSUM") as pq,
            tc.tile_pool(name="p2", bufs=2, space="PSUM") as po,
            tc.tile_pool(name="pxs", bufs=1, space="PSUM") as pxs,
            tc.tile_pool(name="ptiny", bufs=1, space="PSUM") as pt,
        ):
            # ---------------- PE warmup ----------------
            # tiny matmuls bridge the DMA startup so the PE is continuously
            # busy into its 3us ramp when real work arrives.
            wz = cpool.tile([1, Co], bf16)
            nc.gpsimd.memset(wz[:, :], 0.0)
            n_warm = CFG["nwarm"]
            # tt: single psum bank shared by warmup and all tiny outs;
            # xtp separate (its accumulation group stays open all round and
            # a start=True in the same bank would pending-zero it).
            tt = pt.tile([CH, 264], f32, tag="tt")
            xtq = pxs.tile([125, Co], f32, tag="xt")
            warm = tt[0:1, 0:Co]
            xtp = xtq[:, :]
            xm_p = tt[0:64, 64:89]
            t1p = tt[0:65, 89:114]
            simp = tt[0:25, 114:139]
            bdcmp = tt[0:CH, 139:264]
            for _ in range(n_warm):
                nc.tensor.matmul(
                    out=warm, lhsT=wz[:, 0:1], rhs=wz[:, :],
                    start=True, stop=True,
                )

            # ---------------- constants ----------------
            ct = cpool.tile([128, CF], bf16)
            # p1: wstack only (feeds the first step1 group) — smallest first
            nc.sync.dma_start(out=ct[:, 0:256], in_=c_ext[:, 0:256])

            wstack = ct[:, 0:256]           # [W_eff cat | I64] x2 halves
            selc = ct[0:V, 256:886]         # 5 tau-selectors [25, 126]
            bgab3f = ct[0:CH, 886:1261]     # A+B blockdiag bg + bias row
            mvcat = ct[0:C, 1261:1326]      # [M^T | W_ph^T b_th]
            obd = ct[0:125, 1326:1351]      # tau-sum selector * (1/T)

            # y_sb double buffer; row 125 = const b_eff row via DMA, filled
            # once per buffer and never overwritten (copies write 0:125).
            y_bufs = [
                ypool.tile([CH, YW], bf16, tag="y", name=f"ybuf{i}")
                for i in range(2)
            ]

            # xm_aug buffers; row 64 = ones (memset once per buffer)
            xm_bufs = [
                xmpool.tile([65, V], bf16, tag="xm", name=f"xmbuf{i}")
                for i in range(2)
            ]

            def consts_p2():
                nc.gpsimd.dma_start(out=ct[:, 256:CF], in_=c_ext[:, 256:CF])
                for xb in xm_bufs:
                    nc.gpsimd.memset(xb[64:65, :], 1.0)

            def fill_yrow(i, qa, qb):
                """Fill y buffer i's bias row in two half-DMAs (the cost
                model charges per-partition bytes, so this single-partition
                row is expensive — split across two queues)."""
                yb = y_bufs[i]
                h = YW // 2
                qa.dma_start(out=yb[125:126, 0:h], in_=yr_ext[:, 0:h])
                qb.dma_start(out=yb[125:126, h:YW], in_=yr_ext[:, h:YW])

            # ---------------- per-sample phases ----------------
            def phase_a_start(n):
                """Allocate tiles + x DMA for sample n."""
                x_sb = xpool.tile([128, XW], bf16, tag="x")
                if n == 0:
                    # piece boundaries must be 4-byte aligned (even bf16 cols)
                    cuts = [0, 250, 876, 1876, 2812, XW]
                    qs = [nc.sync, nc.gpsimd, nc.sync, nc.gpsimd, nc.sync]
                else:
                    cuts = [0, 1876, XW]
                    qs = [nc.sync, nc.gpsimd]
                for q, (lo, hi) in zip(qs, zip(cuts, cuts[1:])):
                    q.dma_start(out=x_sb[:, lo:hi], in_=x_ext[n][:, lo:hi])
                return {"x": x_sb, "y": y_bufs[n % 2], "n": n}

            def emit_xt(ctx, lo, hi):
                """x-sum accumulation matmuls for pairs [lo, hi), K=128."""
                x_sb = ctx["x"]
                for pr in range(lo, hi):
                    nc.tensor.matmul(
                        out=xtp,
                        lhsT=x_sb[:, pr * 125 : (pr + 1) * 125],
                        rhs=wstack[:, 192:256],
                        start=(pr == 0),
                        stop=(pr == NPR - 1),
                    )

            def emit_group(ctx, g, mid=None, split=False, ceng="a"):
                """step1 psum group g: 4 y-matmuls, then psum->sbuf copy.
                A matmul out may not cross a 512-f32 psum bank boundary, so
                chunk j sits at column (j//2)*512 + (j%2)*192.
                split=True copies the two banks on BOTH engines in parallel
                (halves the copy latency when PE is pq-recycle-bound)."""
                x_sb, y_sb = ctx["x"], ctx["y"]
                yp = pq.tile([125, 1024], f32, tag="p1")
                # chunk order (4g, 4g+2, 4g+1, 4g+3): each psum bank's
                # accumulation group then uses a single partition half (a
                # group spanning two tile_positions breaks the HW path).
                # y_sb stores chunks in this interleaved slot order; step2
                # remaps columns via _ycol.
                for j in range(G1):
                    ch = G1 * g + (0, 2, 1, 3)[j]
                    pr, half = ch >> 1, ch & 1
                    col = (j // 2) * 512 + (j % 2) * W2
                    nc.tensor.matmul(
                        out=yp[:, col : col + W2],
                        lhsT=x_sb[64 * half : 64 * half + 64,
                                  pr * 125 : (pr + 1) * 125],
                        rhs=wstack[64 * half : 64 * half + 64, 0:W2],
                        start=(j % 2 == 0),
                        stop=(j % 2 == 1),
                    )
                if mid is not None:
                    mid()  # latency-critical ops enqueue ahead of the copy
                c0 = g * G1 * W2
                # GPSIMD cannot access PSUM, so psum->sbuf copies are split
                # across ACT and DVE only.
                if split:
                    ea, eb = ((nc.scalar.copy, nc.vector.tensor_copy)
                              if ceng == "a" else
                              (nc.vector.tensor_copy, nc.scalar.copy))
                    ea(out=y_sb[0:125, c0 : c0 + 2 * W2],
                       in_=yp[:, 0 : 2 * W2])
                    eb(out=y_sb[0:125, c0 + 2 * W2 : c0 + 4 * W2],
                       in_=yp[:, 512 : 512 + 2 * W2])
                    return
                dst = y_sb[
                    0:125, c0 : c0 + G1 * W2
                ].rearrange("p (b w) -> p b w", w=2 * W2)
                src = yp[:, :].rearrange("p (b w) -> p b w", w=512)[
                    :, :, 0 : 2 * W2
                ]
                if ceng == "a":
                    nc.scalar.copy(out=dst, in_=src)
                else:
                    nc.vector.tensor_copy(out=dst, in_=src)

            def tiny_steps(ctx):
                """Mean/softmax/bd3 chain as 5 steps; weave each between PE
                group emissions so cross-engine hops never stall the PE."""
                n = ctx["n"]
                xm_aug = xm_bufs[n % 2]

                def s0():  # xta copy (DVE)
                    xta = spool.tile([125, C], bf16, tag="xta")
                    nc.vector.tensor_copy(out=xta[:, :], in_=xtp)
                    ctx["xta"] = xta

                def s1():  # xm = xta^T obd  (obd carries 1/T)
                    nc.tensor.matmul(
                        out=xm_p, lhsT=ctx["xta"][:, :], rhs=obd,
                        start=True, stop=True,
                    )
                    nc.vector.tensor_copy(out=xm_aug[0:C, :], in_=xm_p)

                def s2():  # t1 = [M^T | v]^T xm
                    nc.tensor.matmul(
                        out=t1p, lhsT=mvcat, rhs=xm_aug[0:C, :],
                        start=True, stop=True,
                    )
                    t1sb = spool.tile([65, V], bf16, tag="t1")
                    nc.scalar.copy(out=t1sb[:, :], in_=t1p)
                    ctx["t1"] = t1sb

                def s3():  # sim = [xm;1]^T [t1;r], then softmax -> cm
                    nc.tensor.matmul(
                        out=simp, lhsT=xm_aug[:, :], rhs=ctx["t1"][:, :],
                        start=True, stop=True,
                    )
                    ex = spool.tile([V, V], f32, tag="ex")
                    rs = spool.tile([V, 1], f32, tag="rs")
                    nc.scalar.activation(
                        out=ex[:, :], in_=simp, func=ACTF.Exp,
                        accum_out=rs[:, :],
                    )
                    rr = spool.tile([V, 1], f32, tag="rr")
                    nc.vector.reciprocal(out=rr[:, :], in_=rs[:, :])
                    cmb = spool.tile([V, V], bf16, tag="cm")
                    nc.vector.tensor_scalar(
                        out=cmb[:, :], in0=ex[:, :],
                        scalar1=rr[:, 0:1], scalar2=None, op0=ALU.mult,
                    )
                    ctx["cmb"] = cmb

                def s4():  # blockdiag(cm) via 5 selector matmuls, then
                    # bd3 = bgab3 (static A+B blockdiags + bias row) + tiled
                    # blockdiag(cm) in one wide vector op.
                    for tau in range(5):
                        nc.tensor.matmul(
                            out=bdcmp[:, 25 * tau : 25 * tau + 25],
                            lhsT=selc[:, tau * CH : (tau + 1) * CH],
                            rhs=ctx["cmb"][:, :],
                            start=True, stop=True,
                        )
                    bd3 = spool.tile([CH, S * 125], bf16, tag="bd3")
                    nc.vector.tensor_tensor(
                        out=bd3[:, :].rearrange("p (s w) -> p s w", w=125),
                        in0=bgab3f.rearrange("p (s w) -> p s w", w=125),
                        in1=bdcmp[:, None, :].broadcast_to([CH, S, 125]),
                        op=ALU.add,
                    )
                    ctx["bd3"] = bd3

                return [s0, s1, s2, s3, s4]

            def phase_b_state(n, fine_tail=False, flip=False):
                o_sb = opool.tile([125, NCH * Co], bf16, tag="o")
                # (group_size, copy_engine); engines from CFG["beng"]
                be = CFG["beng"]
                sizes = ([8] * 7 + [2, 1, 1]) if fine_tail else [8] * 7 + [4]
                plan = [(s, be[i % len(be)]) for i, s in enumerate(sizes)]
                dmas = {2: (0, 24), 5: (24, 48)}
                if fine_tail:
                    dmas.update({6: (48, 56), 7: (56, 58), 8: (58, 59),
                                 9: (59, 60)})
                else:
                    dmas.update({7: (48, 60)})
                ch0s, c = [], 0
                for nch, _ in plan:
                    ch0s.append(c)
                    c += nch
                return {"n": n, "o": o_sb, "plan": plan, "dmas": dmas,
                        "ch0s": ch0s}

            def phase_b_group(bs, ctx, g):
                """step2 psum group g: s-accumulated matmuls + copy + DMA."""
                n, o_sb = bs["n"], bs["o"]
                y_sb, bd3 = ctx["y"], ctx["bd3"]
                nch, eng = bs["plan"][g]
                ch0 = bs["ch0s"][g]
                op = po.tile([125, G2 * Co], f32, tag="p2")
                for s in range(S):
                    for j in range(nch):
                        ch = ch0 + j
                        nc.tensor.matmul(
                            out=op[:, j * Co : (j + 1) * Co],
                            lhsT=bd3[:, s * 125 : (s + 1) * 125],
                            rhs=y_sb[:, _ycol(ch) + s * Co
                                     : _ycol(ch) + (s + 1) * Co],
                            start=(s == 0 and j == 0),
                            stop=(s == S - 1 and j == nch - 1),
                        )
                dst = o_sb[:, ch0 * Co : (ch0 + nch) * Co]
                src = op[:, 0 : nch * Co]
                if eng == "a":
                    nc.scalar.copy(out=dst, in_=src)
                else:
                    nc.vector.tensor_copy(out=dst, in_=src)
                if g in bs["dmas"]:
                    lo, hi = bs["dmas"][g]
                    q = {6: nc.gpsimd, 7: nc.sync, 8: nc.gpsimd,
                         9: nc.scalar}.get(g, nc.sync)
                    q.dma_start(
                        out=out_ext[n][:, lo * Co : hi * Co],
                        in_=o_sb[:, lo * Co : hi * Co],
                    )

            # pipeline. Round n emits sample n's 15 step1 groups; sample n's
            # x-sums run one round EARLY (n>=1) so its tiny chain can start
            # at round n g0 and B(n) can begin mid-round, chasing the y
            # copies. B(n) groups left over drain into round n+1's slots.
            ctxs = [phase_a_start(0)]
            ctxs.append(phase_a_start(1))  # x1 up front (buffer B)
            consts_p2()
            if "no_yrow" not in CFG.get("flags", ()):
                fill_yrow(0, nc.sync, nc.gpsimd)

            # round 0: xt(0) at g1-6 paced by x arrival; tiny(0) at g7-11;
            # xt(1) at g9-14; B(0) j0-1 at g13-14.
            bs0 = phase_b_state(0)
            st = tiny_steps(ctxs[0])
            xs1 = CFG["xt1_start"]
            t0s = CFG["tiny0_start"]
            r0b = list(CFG["r0bslots"])
            FL = CFG.get("flags", ())
            for g in range(15):
                midf = st[g - t0s] if t0s <= g <= t0s + 4 else None
                if "no_tiny" in FL:
                    midf = None
                emit_group(ctxs[0], g, mid=midf, split=(g in CFG["split0"]),
                           ceng=CFG["ceng0"][g])
                if "no_xt" not in FL:
                    if CFG["r0xt6"]:
                        if 1 <= g <= 5:
                            emit_xt(ctxs[0], (g - 1) * 6, g * 6)
                    elif 1 <= g <= 6:
                        emit_xt(ctxs[0], (g - 1) * 5, g * 5)
                    if "no_tiny" not in FL and xs1 <= g <= xs1 + 5:
                        emit_xt(ctxs[1], (g - xs1) * 5, (g - xs1 + 1) * 5)
                if "no_b0" in FL:
                    continue
                while r0b and r0b[0] == g:
                    r0b.pop(0)
                    phase_b_group(bs0, ctxs[0],
                                  len(CFG["r0bslots"]) - len(r0b) - 1)

            # rounds 1-2: tiny(n) at g0-4, B(n-1) remainder at odd g,
            # B(n) j0-2 at g12-14, xt(n+1) at g9-14.
            trunc = CFG.get("trunc", 4)
            bprev = bs0
            for n in (1, 2):
                if n >= trunc:
                    break
                bs = phase_b_state(n, flip=True)
                st = tiny_steps(ctxs[n])
                ctxs.append(phase_a_start(n + 1))
                fill_yrow(1, nc.sync, nc.gpsimd) if n == 1 else None
                nb_prev = (len(CFG["r0bslots"]) if n == 1 else
                           sum(1 for _, w in CFG["bslots12"] if w == 1))
                jnext = [nb_prev, 0]  # next group idx for [B(n-1), B(n)]
                slot = {}
                for g, who in CFG["bslots12"]:
                    slot.setdefault(g, []).append(who)
                t12 = CFG["tiny12_start"]
                for g in range(15):
                    midf = st[g - t12] if t12 <= g <= t12 + 4 else None
                    emit_group(ctxs[n], g, mid=midf,
                               split=(g in CFG["split12"]),
                               ceng=CFG["ceng12"][g])
                    for who in slot.get(g, []):
                        if who == 0:
                            phase_b_group(bprev, ctxs[n - 1], jnext[0])
                            jnext[0] += 1
                        else:
                            phase_b_group(bs, ctxs[n], jnext[1])
                            jnext[1] += 1
                    if 9 <= g <= 14:
                        emit_xt(ctxs[n + 1], (g - 9) * 5, (g - 8) * 5)
                bprev = bs
            # round 3: tiny(3) at g0-4; B(2) remainder at even g; B(3) from
            # g5 chasing its own y copies; fine tail drains after the loop.
            if trunc >= 4:
                bs3 = phase_b_state(3, fine_tail=True)
                st3 = tiny_steps(ctxs[3])
                jnext3 = [sum(1 for _, w in CFG["bslots12"] if w == 1), 0]
                slot3 = {}
                for g, who in CFG["r3plan"]:
                    slot3.setdefault(g, []).append(who)
                for g in range(15):
                    midf = st3[g] if g <= 4 else None
                    emit_group(ctxs[3], g, mid=midf,
                               split=(g in CFG["split3"]),
                               ceng=CFG["ceng3"][g])
                    for who in slot3.get(g, []):
                        if who == 2:
                            phase_b_group(bprev, ctxs[2], jnext3[0])
                            jnext3[0] += 1
                        else:
                            phase_b_group(bs3, ctxs[3], jnext3[1])
                            jnext3[1] += 1
                for j in range(jnext3[1], len(bs3["plan"])):
                    phase_b_group(bs3, ctxs[3], j)

    nc.finalize()
    return nc


def _prep_consts(A, B, W_theta, b_theta, W_phi, b_phi, W_big, b_big):
    f = np.float32
    ct = np.zeros((128, CF), dtype=f)
    # wstack: [W_eff cat over s | I64], duplicated in both partition halves
    weff = W_big.reshape(S, Co, S, C).sum(axis=0)  # [co, s, c]
    wst = np.zeros((64, 256), dtype=f)
    wst[:, 0:192] = weff.transpose(2, 1, 0).reshape(C, S * Co)
    wst[:, 192:256] = np.eye(C, dtype=f)
    ct[0:64, 0:256] = wst
    ct[64:128, 0:256] = wst
    # selector lhsTs: sel_tau[v, p] = 1 iff p == 25*tau + v
    for tau in range(5):
        for v in range(V):
            ct[v, 256 + tau * CH + 25 * tau + v] = 1.0
    # bd3 background: blockdiag(A_s+B_s) per s-block + bias row in s=0
    AB = (A + B).astype(f)
    for sb in range(S):
        for tau in range(5):
            r0, c0 = 25 * tau, 886 + 125 * sb + 25 * tau
            ct[r0 : r0 + 25, c0 : c0 + 25] = AB[sb]
    ct[125, 886:1011] = 1.0
    # Mv_cat = [ (W_th^T W_ph)^T | W_ph^T b_th ] = [ W_ph^T W_th | W_ph^T b_th ]
    ct[0:C, 1261:1325] = W_phi.T @ W_theta
    ct[0:C, 1325] = W_phi.T @ b_theta
    # obd: tau-sum selector with 1/T folded in
    ct[0:125, 1326:1351] = np.tile(np.eye(V, dtype=f), (5, 1)) / T

    b_eff = b_big.reshape(S, Co).sum(axis=0)
    yrow = np.zeros((NCH, W2), dtype=f)
    yrow[:, 0:Co] = b_eff
    return {
        "consts": ct.astype(ml_dtypes.bfloat16),
        "yrow": yrow.reshape(1, YW).astype(ml_dtypes.bfloat16),
    }


def _prep_x(x):
    bf = ml_dtypes.bfloat16
    # pair packing: chunk 2j -> partitions 0:64, chunk 2j+1 -> 64:128
    xv = x.reshape(N, C, NPR, 2, 125)
    xp = xv.transpose(0, 3, 1, 2, 4).reshape(N, 128, XW).astype(bf)
    return xp


def kernel(x, A, B, W_theta, b_theta, W_phi, b_phi, W_big, b_big, _profile=None):
    _import_concourse()
    from concourse.bass_utils import run_bass_kernel_spmd

    x = np.asarray(x, dtype=np.float32)
    xp = _prep_x(x)

    consts = _prep_consts(
        np.asarray(A, np.float32), np.asarray(B, np.float32),
        np.asarray(W_theta, np.float32), np.asarray(b_theta, np.float32),
        np.asarray(W_phi, np.float32), np.asarray(b_phi, np.float32),
        np.asarray(W_big, np.float32), np.asarray(b_big, np.float32),
    )

    if "nc" not in _CACHE:
        _CACHE["nc"] = _build_nc()
    nc = _CACHE["nc"]

    in_maps = []
    for i in range(NCORES):
        m = {"x": np.ascontiguousarray(xp[i * NL : (i + 1) * NL])}
        m.update(consts)
        in_maps.append(m)

    kw = {}
    if _profile:
        kw = dict(trace=True, tmpdir=_profile)
    res = run_bass_kernel_spmd(nc, in_maps, list(range(NCORES)), **kw)

    out = np.empty((N, Co, T, V), dtype=np.float32)
    for i in range(NCORES):
        buf = np.asarray(res.results[i]["out"], dtype=np.float32).reshape(
            NL, 5, V, NCH, Co
        )
        # [n, tau, w, ch, co] -> [n, co, ch, tau, w]
        out[i * NL : (i + 1) * NL] = (
            buf.transpose(0, 4, 3, 1, 2).reshape(NL, Co, T, V)
        )
    if _profile:
        _CACHE["exec_time_ns"] = res.exec_time_ns
    return out


# revision 64
# speedup vs baseline: 1.1140x; 1.0021x over previous
"""AdaptiveGraphConv Trainium2 kernel — 8-core batch-parallel Bass/Tile.

Math (per sample n):
  xm     = mean_t x[n]                                  [C, V]
  sim    = (W_th xm + b_th)^T (W_ph xm + b_ph)          [V, V]
  Cmat   = softmax_w(sim)
  adap_s = A[s] + B[s] + Cmat                           [V, V]
  out[n] = sum_s W_eff_s @ x[n] @_v adap_s + b_eff      [Co, T, V]
where W_eff_s[co,c] = sum_sg W_big[sg*Co+co, s*C+c], b_eff = sum_sg b_big[sg*Co:+Co].

Softmax is invariant to per-row(v) offsets, so
  sim ~ xm^T M xm + 1 (b_th^T W_ph xm)   with M = W_th^T W_ph
and the v-only/constant terms are dropped. Host sends Mv = [M^T | W_ph^T b_th].

Device dataflow (per core, 4 samples). T*V splits into 60 chunks of
(5t, 25v) = 125 elements; chunk PAIRS share free columns with chunk 2j in
partitions 0-63 and 2j+1 in 64-127 ("half" packing):
  step1: matmul(lhsT = x half [64, 125], rhs = wstack half [64, 192])
         -> y chunk [(5t,25v), (s,co)] in PSUM, groups of 4 chunks
         -> y_sb bf16 rows 0:125 (ACT/DVE copies); y_sb row 125 is the
         constant b_eff row (s=0 block), DMA-prefilled once per buffer.
  mean : matmul(lhsT = x pair [128, 125], rhs = [I64;I64]) accumulated over
         30 pairs -> xtp [125, 64] (exact, K=128)
  tiny : xm = xta^T obd (obd carries 1/T); t1 = [M^T|v]^T xm;
         sim = [xm;1]^T [t1;r]; softmax -> cm; blockdiag -> bd3 (bias row
         from static A+B background consts)
  step2: matmul(lhsT = bd3 s-block [126,125], rhs = y chunk [126, 64co])
         accumulated over s -> [125, 64co] -> o_sb bf16 -> DMA (host upcasts)
"""

import numpy as np
import ml_dtypes

N, C, T, V, S, E, Co = 32, 64, 300, 25, 3, 64, 64
NCORES = 8
NL = N // NCORES          # samples per core = 4
CH = 126                  # bd3 partitions: 125 data + bias row
NCH = 60                  # chunks per sample (T/5)
NPR = 30                  # chunk pairs per sample
W2 = S * Co               # 192 = y columns per chunk
XW = NPR * 125            # 3750 = x free size per sample (pair-packed)
G1 = 4                    # step1 chunks per psum group (15 groups)
G2 = 8                    # step2 chunks per psum group
CF = 1352                 # packed consts free size (even cols: 4B row stride)
YW = NCH * W2             # 11520 = y_sb free size
N_WARM = 32               # PE warmup matmuls (bridge DMA cold start)

# schedule tuning knobs (see _build_nc); sweepable via kernel.CFG.update()
# bslots12: R1/R2 slot plan — (g, 0) -> next B(n-1) group, (g, 1) -> next
# B(n) group. r3plan: (g, 2) -> next B2 group, (g, 3) -> next B3 group.
CFG = {
    "nwarm": 26,
    "split0": (),         # R0 groups with split copies
    "split12": (),        # R1/R2 groups with split copies
    "split3": (),         # R3 groups with split copies
    "xt1_start": 9,       # R0 weave start for xt(1)
    "r0xt6": False,       # xt(0) at 6/group g1-5 (else 5/group g1-6)
    "tiny0_start": 8,     # R0 tiny chain start group
    "tiny12_start": 1,    # R1/R2 tiny chain start group
    "ceng0": "avavavaavvaavav",    # R0 step1 copy engines per group
    "ceng12": "avavvaavavaavva",   # R1/R2 step1 copy engines
    "ceng3": "avavavavavvaava",    # R3 step1 copy engines
    "beng": "avavavavaa",          # step2 copy engines per group idx
    "r0bslots": (13, 14, 14),   # R0 slots for B0 j0..k-1
    "bslots12": ((1, 0), (3, 0), (5, 0), (8, 0), (8, 0),
                 (9, 1), (10, 1), (12, 1)),
    "r3plan": ((3, 2), (5, 2), (5, 2), (5, 2), (6, 2), (6, 3),
               (6, 3), (11, 3), (12, 3), (12, 3), (14, 3)),
}

_YPERM = (0, 2, 1, 3)  # stored slot of chunk ch within its group


def _ycol(ch):
    """y_sb column of chunk ch (chunks stored group-interleaved)."""
    return ((ch >> 2) * 4 + _YPERM[ch & 3]) * W2


_CACHE = {}


def _import_concourse():
    try:
        import concourse  # noqa: F401
    except ImportError:
        import sys

        for p in ("/opt/trn_rl_repo", "/root/.axon_site/_ro/trn_rl_repo"):
            if p not in sys.path:
                sys.path.insert(0, p)


def _build_nc():
    _import_concourse()
    import concourse.bass as bass
    import concourse.bacc as bacc
    import concourse.mybir as mybir
    from concourse import tile

    dt = mybir.dt
    f32, bf16 = dt.float32, dt.bfloat16
    ALU = mybir.AluOpType
    ACTF = mybir.ActivationFunctionType

    nc = bacc.Bacc(None, target_bir_lowering=False)

    x_ext = nc.declare_dram_parameter("x", [NL, 128, XW], bf16, isOutput=False)
    c_ext = nc.declare_dram_parameter("consts", [128, CF], bf16, isOutput=False)
    yr_ext = nc.declare_dram_parameter("yrow", [1, YW], bf16, isOutput=False)
    out_ext = nc.declare_dram_parameter(
        "out", [NL, 125, NCH * Co], bf16, isOutput=True
    )

    with tile.TileContext(nc) as tc:
        with (
            tc.tile_pool(name="const", bufs=1) as cpool,
            tc.tile_pool(name="xin", bufs=2) as xpool,
            tc.tile_pool(name="y", bufs=2) as ypool,
            tc.tile_pool(name="osb", bufs=2) as opool,
            tc.tile_pool(name="small", bufs=2) as spool,
            tc.tile_pool(name="xmaug", bufs=2) as xmpool,
            tc.tile_pool(name="p1", bufs=2, space="PSUM") as pq,
            tc.tile_pool(name="p2", bufs=2, space="PSUM") as po,
            tc.tile_pool(name="pxs", bufs=1, space="PSUM") as pxs,
            tc.tile_pool(name="ptiny", bufs=1, space="PSUM") as pt,
        ):
            # ---------------- PE warmup ----------------
            # tiny matmuls bridge the DMA startup so the PE is continuously
            # busy into its 3us ramp when real work arrives.
            wz = cpool.tile([1, Co], bf16)
            nc.gpsimd.memset(wz[:, :], 0.0)
            n_warm = CFG["nwarm"]
            # tt: single psum bank shared by warmup and all tiny outs;
            # xtp separate (its accumulation group stays open all round and
            # a start=True in the same bank would pending-zero it).
            tt = pt.tile([CH, 264], f32, tag="tt")
            xtq = pxs.tile([125, Co], f32, tag="xt")
            warm = tt[0:1, 0:Co]
            xtp = xtq[:, :]
            xm_p = tt[0:64, 64:89]
            t1p = tt[0:65, 89:114]
            simp = tt[0:25, 114:139]
            bdcmp = tt[0:CH, 139:264]
            for _ in range(n_warm):
                nc.tensor.matmul(
                    out=warm, lhsT=wz[:, 0:1], rhs=wz[:, :],
                    start=True, stop=True,
                )

            # ---------------- constants ----------------
            ct = cpool.tile([128, CF], bf16)
            # p1: wstack only (feeds the first step1 group) — smallest first
            nc.sync.dma_start(out=ct[:, 0:256], in_=c_ext[:, 0:256])

            wstack = ct[:, 0:256]           # [W_eff cat | I64] x2 halves
            selc = ct[0:V, 256:886]         # 5 tau-selectors [25, 126]
            bgab3f = ct[0:CH, 886:1261]     # A+B blockdiag bg + bias row
            mvcat = ct[0:C, 1261:1326]      # [M^T | W_ph^T b_th]
            obd = ct[0:125, 1326:1351]      # tau-sum selector * (1/T)

            # y_sb double buffer; row 125 = const b_eff row via DMA, filled
            # once per buffer and never overwritten (copies write 0:125).
            y_bufs = [
                ypool.tile([CH, YW], bf16, tag="y", name=f"ybuf{i}")
                for i in range(2)
            ]

            # xm_aug buffers; row 64 = ones (memset once per buffer)
            xm_bufs = [
                xmpool.tile([65, V], bf16, tag="xm", name=f"xmbuf{i}")
                for i in range(2)
            ]

            def consts_p2():
                nc.gpsimd.dma_start(out=ct[:, 256:CF], in_=c_ext[:, 256:CF])
                for xb in xm_bufs:
                    nc.gpsimd.memset(xb[64:65, :], 1.0)

            def fill_yrow(i, qa, qb):
                """Fill y buffer i's bias row in two half-DMAs (the cost
                model charges per-partition bytes, so this single-partition
                row is expensive — split across two queues)."""
                yb = y_bufs[i]
                h = YW // 2
                qa.dma_start(out=yb[125:126, 0:h], in_=yr_ext[:, 0:h])
                qb.dma_start(out=yb[125:126, h:YW], in_=yr_ext[:, h:YW])

            # ---------------- per-sample phases ----------------
            def phase_a_start(n):
                """Allocate tiles + x DMA for sample n."""
                x_sb = xpool.tile([128, XW], bf16, tag="x")
                if n == 0:
                    # piece boundaries must be 4-byte aligned (even bf16 cols)
                    cuts = [0, 250, 876, 1876, 2812, XW]
                    qs = [nc.sync, nc.gpsimd, nc.sync, nc.gpsimd, nc.sync]
                else:
                    cuts = [0, 1876, XW]
                    qs = [nc.sync, nc.gpsimd]
                for q, (lo, hi) in zip(qs, zip(cuts, cuts[1:])):
                    q.dma_start(out=x_sb[:, lo:hi], in_=x_ext[n][:, lo:hi])
                return {"x": x_sb, "y": y_bufs[n % 2], "n": n}

            def emit_xt(ctx, lo, hi):
                """x-sum accumulation matmuls for pairs [lo, hi), K=128."""
                x_sb = ctx["x"]
                for pr in range(lo, hi):
                    nc.tensor.matmul(
                        out=xtp,
                        lhsT=x_sb[:, pr * 125 : (pr + 1) * 125],
                        rhs=wstack[:, 192:256],
                        start=(pr == 0),
                        stop=(pr == NPR - 1),
                    )

            def emit_group(ctx, g, mid=None, split=False, ceng="a"):
                """step1 psum group g: 4 y-matmuls, then psum->sbuf copy.
                A matmul out may not cross a 512-f32 psum bank boundary, so
                chunk j sits at column (j//2)*512 + (j%2)*192.
                split=True copies the two banks on BOTH engines in parallel
                (halves the copy latency when PE is pq-recycle-bound)."""
                x_sb, y_sb = ctx["x"], ctx["y"]
                yp = pq.tile([125, 1024], f32, tag="p1")
                # chunk order (4g, 4g+2, 4g+1, 4g+3): each psum bank's
                # accumulation group then uses a single partition half (a
                # group spanning two tile_positions breaks the HW path).
                # y_sb stores chunks in this interleaved slot order; step2
                # remaps columns via _ycol.
                for j in range(G1):
                    ch = G1 * g + (0, 2, 1, 3)[j]
                    pr, half = ch >> 1, ch & 1
                    col = (j // 2) * 512 + (j % 2) * W2
                    nc.tensor.matmul(
                        out=yp[:, col : col + W2],
                        lhsT=x_sb[64 * half : 64 * half + 64,
                                  pr * 125 : (pr + 1) * 125],
                        rhs=wstack[64 * half : 64 * half + 64, 0:W2],
                        start=(j % 2 == 0),
                        stop=(j % 2 == 1),
                    )
                if mid is not None:
                    mid()  # latency-critical ops enqueue ahead of the copy
                c0 = g * G1 * W2
                # GPSIMD cannot access PSUM, so psum->sbuf copies are split
                # across ACT and DVE only.
                if split:
                    ea, eb = ((nc.scalar.copy, nc.vector.tensor_copy)
                              if ceng == "a" else
                              (nc.vector.tensor_copy, nc.scalar.copy))
                    ea(out=y_sb[0:125, c0 : c0 + 2 * W2],
                       in_=yp[:, 0 : 2 * W2])
                    eb(out=y_sb[0:125, c0 + 2 * W2 : c0 + 4 * W2],
                       in_=yp[:, 512 : 512 + 2 * W2])
                    return
                dst = y_sb[
                    0:125, c0 : c0 + G1 * W2
                ].rearrange("p (b w) -> p b w", w=2 * W2)
                src = yp[:, :].rearrange("p (b w) -> p b w", w=512)[
                    :, :, 0 : 2 * W2
                ]
                if ceng == "a":
                    nc.scalar.copy(out=dst, in_=src)
                else:
                    nc.vector.tensor_copy(out=dst, in_=src)

            def tiny_steps(ctx):
                """Mean/softmax/bd3 chain as 5 steps; weave each between PE
                group emissions so cross-engine hops never stall the PE."""
                n = ctx["n"]
                xm_aug = xm_bufs[n % 2]

                def s0():  # xta copy (DVE)
                    xta = spool.tile([125, C], bf16, tag="xta")
                    nc.vector.tensor_copy(out=xta[:, :], in_=xtp)
                    ctx["xta"] = xta

                def s1():  # xm = xta^T obd  (obd carries 1/T)
                    nc.tensor.matmul(
                        out=xm_p, lhsT=ctx["xta"][:, :], rhs=obd,
                        start=True, stop=True,
                    )
                    nc.vector.tensor_copy(out=xm_aug[0:C, :], in_=xm_p)

                def s2():  # t1 = [M^T | v]^T xm
                    nc.tensor.matmul(
                        out=t1p, lhsT=mvcat, rhs=xm_aug[0:C, :],
                        start=True, stop=True,
                    )
                    t1sb = spool.tile([65, V], bf16, tag="t1")
                    nc.scalar.copy(out=t1sb[:, :], in_=t1p)
                    ctx["t1"] = t1sb

                def s3():  # sim = [xm;1]^T [t1;r], then softmax -> cm
                    nc.tensor.matmul(
                        out=simp, lhsT=xm_aug[:, :], rhs=ctx["t1"][:, :],
                        start=True, stop=True,
                    )
                    ex = spool.tile([V, V], f32, tag="ex")
                    rs = spool.tile([V, 1], f32, tag="rs")
                    nc.scalar.activation(
                        out=ex[:, :], in_=simp, func=ACTF.Exp,
                        accum_out=rs[:, :],
                    )
                    rr = spool.tile([V, 1], f32, tag="rr")
                    nc.vector.reciprocal(out=rr[:, :], in_=rs[:, :])
                    cmb = spool.tile([V, V], bf16, tag="cm")
                    nc.vector.tensor_scalar(
                        out=cmb[:, :], in0=ex[:, :],
                        scalar1=rr[:, 0:1], scalar2=None, op0=ALU.mult,
                    )
                    ctx["cmb"] = cmb

                def s4():  # blockdiag(cm) via 5 selector matmuls, then
                    # bd3 = bgab3 (static A+B blockdiags + bias row) + tiled
                    # blockdiag(cm) in one wide vector op.
                    for tau in range(5):
                        nc.tensor.matmul(
                            out=bdcmp[:, 25 * tau : 25 * tau + 25],
                            lhsT=selc[:, tau * CH : (tau + 1) * CH],
                            rhs=ctx["cmb"][:, :],
                            start=True, stop=True,
                        )
                    bd3 = spool.tile([CH, S * 125], bf16, tag="bd3")
                    nc.vector.tensor_tensor(
                        out=bd3[:, :].rearrange("p (s w) -> p s w", w=125),
                        in0=bgab3f.rearrange("p (s w) -> p s w", w=125),
                        in1=bdcmp[:, None, :].broadcast_to([CH, S, 125]),
                        op=ALU.add,
                    )
                    ctx["bd3"] = bd3

                return [s0, s1, s2, s3, s4]

            def phase_b_state(n, fine_tail=False, flip=False):
                o_sb = opool.tile([125, NCH * Co], bf16, tag="o")
                # (group_size, copy_engine); engines from CFG["beng"]
                be = CFG["beng"]
                sizes = ([8] * 7 + [2, 1, 1]) if fine_tail else [8] * 7 + [4]
                plan = [(s, be[i % len(be)]) for i, s in enumerate(sizes)]
                dmas = {2: (0, 24), 5: (24, 48)}
                if fine_tail:
                    dmas.update({6: (48, 56), 7: (56, 58), 8: (58, 59),
                                 9: (59, 60)})
                else:
                    dmas.update({7: (48, 60)})
                ch0s, c = [], 0
                for nch, _ in plan:
                    ch0s.append(c)
                    c += nch
                return {"n": n, "o": o_sb, "plan": plan, "dmas": dmas,
                        "ch0s": ch0s}

            def phase_b_group(bs, ctx, g):
                """step2 psum group g: s-accumulated matmuls + copy + DMA."""
                n, o_sb = bs["n"], bs["o"]
                y_sb, bd3 = ctx["y"], ctx["bd3"]
                nch, eng = bs["plan"][g]
                ch0 = bs["ch0s"][g]
                op = po.tile([125, G2 * Co], f32, tag="p2")
                for s in range(S):
                    for j in range(nch):
                        ch = ch0 + j
                        nc.tensor.matmul(
                            out=op[:, j * Co : (j + 1) * Co],
                            lhsT=bd3[:, s * 125 : (s + 1) * 125],
                            rhs=y_sb[:, _ycol(ch) + s * Co
                                     : _ycol(ch) + (s + 1) * Co],
                            start=(s == 0 and j == 0),
                            stop=(s == S - 1 and j == nch - 1),
                        )
                dst = o_sb[:, ch0 * Co : (ch0 + nch) * Co]
                src = op[:, 0 : nch * Co]
                if eng == "a":
                    nc.scalar.copy(out=dst, in_=src)
                else:
                    nc.vector.tensor_copy(out=dst, in_=src)
                if g in bs["dmas"]:
                    lo, hi = bs["dmas"][g]
                    q = {6: nc.gpsimd, 7: nc.sync, 8: nc.gpsimd,
                         9: nc.scalar}.get(g, nc.sync)
                    q.dma_start(
                        out=out_ext[n][:, lo * Co : hi * Co],
                        in_=o_sb[:, lo * Co : hi * Co],
                    )

            # pipeline. Round n emits sample n's 15 step1 groups; sample n's
            # x-sums run one round EARLY (n>=1) so its tiny chain can start
            # at round n g0 and B(n) can begin mid-round, chasing the y
            # copies. B(n) groups left over drain into round n+1's slots.
            ctxs = [phase_a_start(0)]
            ctxs.append(phase_a_start(1))  # x1 up front (buffer B)
            consts_p2()
            if "no_yrow" not in CFG.get("flags", ()):
                fill_yrow(0, nc.sync, nc.gpsimd)

            # round 0: xt(0) at g1-6 paced by x arrival; tiny(0) at g7-11;
            # xt(1) at g9-14; B(0) j0-1 at g13-14.
            bs0 = phase_b_state(0)
            st = tiny_steps(ctxs[0])
            xs1 = CFG["xt1_start"]
            t0s = CFG["tiny0_start"]
            r0b = list(CFG["r0bslots"])
            FL = CFG.get("flags", ())
            for g in range(15):
                midf = st[g - t0s] if t0s <= g <= t0s + 4 else None
                if "no_tiny" in FL:
                    midf = None
                emit_group(ctxs[0], g, mid=midf, split=(g in CFG["split0"]),
                           ceng=CFG["ceng0"][g])
                if "no_xt" not in FL:
                    if CFG["r0xt6"]:
                        if 1 <= g <= 5:
                            emit_xt(ctxs[0], (g - 1) * 6, g * 6)
                    elif 1 <= g <= 6:
                        emit_xt(ctxs[0], (g - 1) * 5, g * 5)
                    if "no_tiny" not in FL and xs1 <= g <= xs1 + 5:
                        emit_xt(ctxs[1], (g - xs1) * 5, (g - xs1 + 1) * 5)
                if "no_b0" in FL:
                    continue
                while r0b and r0b[0] == g:
                    r0b.pop(0)
                    phase_b_group(bs0, ctxs[0],
                                  len(CFG["r0bslots"]) - len(r0b) - 1)

            # rounds 1-2: tiny(n) at g0-4, B(n-1) remainder at odd g,
            # B(n) j0-2 at g12-14, xt(n+1) at g9-14.
            trunc = CFG.get("trunc", 4)
            bprev = bs0
            for n in (1, 2):
                if n >= trunc:
                    break
                bs = phase_b_state(n, flip=True)
                st = tiny_steps(ctxs[n])
                ctxs.append(phase_a_start(n + 1))
                fill_yrow(1, nc.sync, nc.gpsimd) if n == 1 else None
                nb_prev = (len(CFG["r0bslots"]) if n == 1 else
                           sum(1 for _, w in CFG["bslots12"] if w == 1))
                jnext = [nb_prev, 0]  # next group idx for [B(n-1), B(n)]
                slot = {}
                for g, who in CFG["bslots12"]:
                    slot.setdefault(g, []).append(who)
                t12 = CFG["tiny12_start"]
                for g in range(15):
                    midf = st[g - t12] if t12 <= g <= t12 + 4 else None
                    emit_group(ctxs[n], g, mid=midf,
                               split=(g in CFG["split12"]),
                               ceng=CFG["ceng12"][g])
                    for who in slot.get(g, []):
                        if who == 0:
                            phase_b_group(bprev, ctxs[n - 1], jnext[0])
                            jnext[0] += 1
                        else:
                            phase_b_group(bs, ctxs[n], jnext[1])
                            jnext[1] += 1
                    if 9 <= g <= 14:
                        emit_xt(ctxs[n + 1], (g - 9) * 5, (g - 8) * 5)
                bprev = bs
            # round 3: tiny(3) at g0-4; B(2) remainder at even g; B(3) from
            # g5 chasing its own y copies; fine tail drains after the loop.
            if trunc >= 4:
                bs3 = phase_b_state(3, fine_tail=True)
                st3 = tiny_steps(ctxs[3])
                jnext3 = [sum(1 for _, w in CFG["bslots12"] if w == 1), 0]
                slot3 = {}
                for g, who in CFG["r3plan"]:
                    slot3.setdefault(g, []).append(who)
                for g in range(15):
                    midf = st3[g] if g <= 4 else None
                    emit_group(ctxs[3], g, mid=midf,
                               split=(g in CFG["split3"]),
                               ceng=CFG["ceng3"][g])
                    for who in slot3.get(g, []):
                        if who == 2:
                            phase_b_group(bprev, ctxs[2], jnext3[0])
                            jnext3[0] += 1
                        else:
                            phase_b_group(bs3, ctxs[3], jnext3[1])
                            jnext3[1] += 1
                for j in range(jnext3[1], len(bs3["plan"])):
                    phase_b_group(bs3, ctxs[3], j)

    nc.finalize()
    return nc


def _prep_consts(A, B, W_theta, b_theta, W_phi, b_phi, W_big, b_big):
    f = np.float32
    ct = np.zeros((128, CF), dtype=f)
    # wstack: [W_eff cat over s | I64], duplicated in both partition halves
    weff = W_big.reshape(S, Co, S, C).sum(axis=0)  # [co, s, c]
    wst = np.zeros((64, 256), dtype=f)
    wst[:, 0:192] = weff.transpose(2, 1, 0).reshape(C, S * Co)
    wst[:, 192:256] = np.eye(C, dtype=f)
    ct[0:64, 0:256] = wst
    ct[64:128, 0:256] = wst
    # selector lhsTs: sel_tau[v, p] = 1 iff p == 25*tau + v
    for tau in range(5):
        for v in range(V):
            ct[v, 256 + tau * CH + 25 * tau + v] = 1.0
    # bd3 background: blockdiag(A_s+B_s) per s-block + bias row in s=0
    AB = (A + B).astype(f)
    for sb in range(S):
        for tau in range(5):
            r0, c0 = 25 * tau, 886 + 125 * sb + 25 * tau
            ct[r0 : r0 + 25, c0 : c0 + 25] = AB[sb]
    ct[125, 886:1011] = 1.0
    # Mv_cat = [ (W_th^T W_ph)^T | W_ph^T b_th ] = [ W_ph^T W_th | W_ph^T b_th ]
    ct[0:C, 1261:1325] = W_phi.T @ W_theta
    ct[0:C, 1325] = W_phi.T @ b_theta
    # obd: tau-sum selector with 1/T folded in
    ct[0:125, 1326:1351] = np.tile(np.eye(V, dtype=f), (5, 1)) / T

    b_eff = b_big.reshape(S, Co).sum(axis=0)
    yrow = np.zeros((NCH, W2), dtype=f)
    yrow[:, 0:Co] = b_eff
    return {
        "consts": ct.astype(ml_dtypes.bfloat16),
        "yrow": yrow.reshape(1, YW).astype(ml_dtypes.bfloat16),
    }


def _prep_x(x):
    bf = ml_dtypes.bfloat16
    # pair packing: chunk 2j -> partitions 0:64, chunk 2j+1 -> 64:128
    xv = x.reshape(N, C, NPR, 2, 125)
    xp = xv.transpose(0, 3, 1, 2, 4).reshape(N, 128, XW).astype(bf)
    return xp


def kernel(x, A, B, W_theta, b_theta, W_phi, b_phi, W_big, b_big, _profile=None):
    _import_concourse()
    from concourse.bass_utils import run_bass_kernel_spmd

    x = np.asarray(x, dtype=np.float32)
    xp = _prep_x(x)

    consts = _prep_consts(
        np.asarray(A, np.float32), np.asarray(B, np.float32),
        np.asarray(W_theta, np.float32), np.asarray(b_theta, np.float32),
        np.asarray(W_phi, np.float32), np.asarray(b_phi, np.float32),
        np.asarray(W_big, np.float32), np.asarray(b_big, np.float32),
    )

    if "nc" not in _CACHE:
        _CACHE["nc"] = _build_nc()
    nc = _CACHE["nc"]

    in_maps = []
    for i in range(NCORES):
        m = {"x": np.ascontiguousarray(xp[i * NL : (i + 1) * NL])}
        m.update(consts)
        in_maps.append(m)

    kw = {}
    if _profile:
        kw = dict(trace=True, tmpdir=_profile)
    res = run_bass_kernel_spmd(nc, in_maps, list(range(NCORES)), **kw)

    out = np.empty((N, Co, T, V), dtype=np.float32)
    for i in range(NCORES):
        buf = np.asarray(res.results[i]["out"], dtype=np.float32).reshape(
            NL, 5, V, NCH, Co
        )
        # [n, tau, w, ch, co] -> [n, co, ch, tau, w]
        out[i * NL : (i + 1) * NL] = (
            buf.transpose(0, 4, 3, 1, 2).reshape(NL, Co, T, V)
        )
    if _profile:
        _CACHE["exec_time_ns"] = res.exec_time_ns
    return out
